# revision 6
# baseline (speedup 1.0000x reference)
"""Bidirectional GRU encoder (Keras reset_after=True) on 8 Trainium2 NeuronCores.

Problem (hardcoded): B=128, T=512, V=32000, D=300, H=128, fp32.
  x = emb[encoder_input]                       # [B,T,300] gather
  out_f, h_f = GRU_fwd(x);  out_b, h_b = GRU_bwd(x reversed)
  return concat([out_f, out_b], -1), h_f, h_b

Sharding: data-parallel over batch, 16 rows per core; both directions run on
every core.  Inside a core everything is feature-major [H=128 partitions,
batch] so the recurrent state feeds the gate matmuls with no transposes.

Device-side plan per core (one Tile program, fully unrolled over T):
  - embedding rows are gathered 128 at a time with indirect DMA into
    [128, 304] row tiles, PE-transposed into x^T tiles [d_chunk, 128]
    (d chunks 128/128/45; the 45th row of chunk 2 is a constant 1 used to
    fold all additive biases into the input-projection matmul),
  - input projections x@W+b for both directions are computed chunk-by-chunk
    (8 timesteps at a time) straight into PSUM; the per-step recurrent
    matmuls U.T @ h accumulate into the same PSUM regions (z, r gates),
  - per step: sigmoid (z,r) -> (rec_h + b_rh) * r -> + x_h -> tanh ->
    h' = hh + z*(h - hh), written directly into the big SBUF output buffer
    which doubles as the recurrent state,
  - every 128 steps finished output columns are PE-transposed and DMA'd to
    DRAM as [b, t, h].
"""

import os
import sys
import functools

import numpy as np

for _p in ("/opt/trn_rl_repo",):
    if _p not in sys.path and os.path.isdir(_p):
        sys.path.insert(0, _p)

import concourse.bass as bass
import concourse.mybir as mybir
import concourse.tile as tile
from concourse import bacc
from concourse.bass import AP, IndirectOffsetOnAxis
from concourse.bass_utils import run_bass_kernel_spmd
from concourse.masks import make_identity

F32 = mybir.dt.float32
I32 = mybir.dt.int32
AF = mybir.ActivationFunctionType
ALU = mybir.AluOpType

# Problem constants
B_FULL, T_FULL, V, D, H = 128, 512, 32000, 300, 128
N_CORES = 8
P = 128  # partitions


def build_program(T: int, Bc: int):
    """Build the single-core Bass/Tile program (SPMD across cores)."""
    TC = P // Bc            # timesteps per psum chunk (8 for Bc=16)
    NT = T * Bc             # total (t, b) positions per direction
    ntile = NT // P         # 128-row tiles of gathered x
    RW = Bc * TC            # region width in psum chunk = 128
    FB = min(128, T)        # output flush block (timesteps)
    DCH = [(0, 128), (128, 128), (256, 45)]  # D-chunks incl. bias row
    assert T % TC == 0 and T % FB == 0 and NT % P == 0

    nc = bacc.Bacc("TRN2", target_bir_lowering=False, debug=False)

    # ---- DRAM tensors -------------------------------------------------
    idx_d = nc.dram_tensor("idx", [P, ntile], I32, kind="ExternalInput")
    emb_d = nc.dram_tensor("emb", [V, D], F32, kind="ExternalInput")
    w_d = {}
    for dname in ("f", "b"):
        for c, (k0, ks) in enumerate(DCH):
            w_d[dname, c] = nc.dram_tensor(f"w{dname}{c}", [ks, 384], F32,
                                           kind="ExternalInput")
    uf_d = nc.dram_tensor("uf", [H, 384], F32, kind="ExternalInput")
    ub_d = nc.dram_tensor("ub", [H, 384], F32, kind="ExternalInput")
    brf_d = nc.dram_tensor("brf", [H, 1], F32, kind="ExternalInput")
    brb_d = nc.dram_tensor("brb", [H, 1], F32, kind="ExternalInput")
    h0_d = nc.dram_tensor("h0", [H, 2 * Bc], F32, kind="ExternalInput")

    out_d = nc.dram_tensor("out", [Bc, T, 2 * H], F32, kind="ExternalOutput")
    hf_d = nc.dram_tensor("hf", [H, Bc], F32, kind="ExternalOutput")
    hb_d = nc.dram_tensor("hb", [H, Bc], F32, kind="ExternalOutput")

    with tile.TileContext(nc) as tc:
        from contextlib import ExitStack
        with ExitStack() as ctx:
            cst = ctx.enter_context(tc.tile_pool(name="cst", bufs=1))
            xtp = ctx.enter_context(tc.tile_pool(name="xtp", bufs=ntile))
            oap = ctx.enter_context(tc.tile_pool(name="oap", bufs=1))
            xrw = ctx.enter_context(tc.tile_pool(name="xrw", bufs=3))
            pwp = ctx.enter_context(tc.tile_pool(name="pwp", bufs=3))
            chp = ctx.enter_context(tc.tile_pool(name="chp", bufs=2, space="PSUM"))
            scp = ctx.enter_context(tc.tile_pool(name="scp", bufs=2, space="PSUM"))
            tpp = ctx.enter_context(tc.tile_pool(name="tpp", bufs=2, space="PSUM"))

            # Bacc's compile passes split multi-sem waits into
            # EventSemaphore instructions (HW allows 1 wait per inst).
            def mm(out, lhsT, rhs, **kw):
                return nc.tensor.matmul(out, lhsT=lhsT, rhs=rhs, **kw)

            def tr(out, in_, identity):
                return nc.tensor.transpose(out, in_, identity)

            # ---- constants / weights into SBUF ------------------------
            ident = cst.tile([P, P], F32, tag="ident")
            make_identity(nc, ident[:])

            idx_sb = cst.tile([P, ntile], I32, tag="idx")
            nc.sync.dma_start(idx_sb[:], idx_d[:])

            w_sb = {}
            for dname in ("f", "b"):
                for c, (k0, ks) in enumerate(DCH):
                    t = cst.tile([ks, 384], F32, tag=f"w{dname}{c}")
                    nc.sync.dma_start(t[:], w_d[dname, c][:])
                    w_sb[dname, c] = t
            uf = cst.tile([H, 384], F32, tag="uf")
            nc.sync.dma_start(uf[:], uf_d[:])
            ub = cst.tile([H, 384], F32, tag="ub")
            nc.sync.dma_start(ub[:], ub_d[:])
            brf = cst.tile([H, 1], F32, tag="brf")
            nc.sync.dma_start(brf[:], brf_d[:])
            brb = cst.tile([H, 1], F32, tag="brb")
            nc.sync.dma_start(brb[:], brb_d[:])
            h0sb = cst.tile([H, 2 * Bc], F32, tag="h0")
            nc.sync.dma_start(h0sb[:], h0_d[:])

            # big persistent buffers
            out_all = oap.tile([P, 2 * NT], F32, tag="out_all")
            oa = out_all[:]
            oa_p = oa.ap[0]  # partition dim [stride, 128]

            def hstate3(s):
                """Combined state [128, 2, 16]: dir-f cols b*T+s, dir-b cols
                NT + b*T + (T-1-s).  Affine per fixed s."""
                if s < 0:
                    return h0sb[:].rearrange("p (d b) -> p d b", d=2)
                off_f = s
                off_b = NT + (T - 1 - s)
                return AP(oa.tensor, oa.offset + off_f,
                          [oa_p, [off_b - off_f, 2], [T, Bc]])

            def hstate_dir(s, d):
                if s < 0:
                    return h0sb[:, d * Bc:(d + 1) * Bc]
                off = s if d == 0 else NT + (T - 1 - s)
                return AP(oa.tensor, oa.offset + off, [oa_p, [T, Bc]])

            # ---- gather + transpose x into feature-major tiles ---------
            # x row n = t*Bc + b ; tile k covers n in [128k, 128k+128)
            xts = {0: [], 1: [], 2: []}
            order = []
            lo, hi = 0, ntile - 1
            while lo <= hi:
                order.append(lo)
                if hi != lo:
                    order.append(hi)
                lo += 1
                hi -= 1

            for k in order:
                xr = xrw.tile([P, 304], F32, tag="xr")
                nc.vector.memset(xr[:, 300:301], 1.0)
                nc.gpsimd.indirect_dma_start(
                    out=xr[:, 0:D], out_offset=None, in_=emb_d[:],
                    in_offset=IndirectOffsetOnAxis(ap=idx_sb[:, k:k + 1], axis=0))
                for c, (k0, ks) in enumerate(DCH):
                    tp = tpp.tile([P, P], F32, tag="tp")
                    tr(tp[0:ks, 0:P], xr[:, k0:k0 + ks], ident[:])
                    xt = xtp.tile([ks, P], F32, tag=f"xt{c}")
                    nc.scalar.copy(xt[:], tp[0:ks, 0:P])
                    xts[c].append((k, xt))
            xt_of = {c: dict(xts[c]) for c in range(3)}

            # ---- psum chunk fill --------------------------------------
            # chunk regions (RW=128 cols each):
            #   0: z_f   1: r_f   2: xh_f   3: z_b   4: r_b   5: xh_b
            def fill_chunk(kc):
                # PSUM has_written semantics: a start=True matmul clears the
                # accumulate-bits of its ENTIRE bank.  So: exactly one
                # start=True per bank per chunk generation (the first matmul
                # touching that bank); everything else start=False —
                # unwritten-since-clear elements are overwritten, written
                # ones accumulate.  The per-step recurrent matmuls later
                # accumulate with start=False onto these regions.
                pf = chp.tile([P, 6 * RW], F32, tag="ch")
                pf6 = pf[:].rearrange("p (r q) -> p r q", r=6)
                for dbase, dname, ktile in ((0, "f", kc), (3, "b", ntile - 1 - kc)):
                    for g in range(3):
                        reg = dbase + g
                        for c, (k0, ks) in enumerate(DCH):
                            first_in_bank = (reg * RW) % 512 == 0 and c == 0
                            mm(pf6[:, reg, :],
                               lhsT=w_sb[dname, c][:, g * 128:(g + 1) * 128],
                               rhs=xt_of[c][ktile][:],
                               start=first_in_bank, stop=(c == 2),
                               skip_group_check=True)
                return pf

            # ---- recurrence -------------------------------------------
            pf = None
            pf6 = None
            for s in range(T):
                kc, dt = divmod(s, TC)
                dtb = TC - 1 - dt
                if dt == 0:
                    pf = fill_chunk(kc)
                    pf6 = pf[:].rearrange("p (r q) -> p r q", r=6)

                hp_f = hstate_dir(s - 1, 0)
                hp_b = hstate_dir(s - 1, 1)
                sc = scp.tile([P, 2 * Bc], F32, tag="sc")

                mm(pf6[:, 0, dt * Bc:(dt + 1) * Bc], lhsT=uf[:, 0:128],
                   rhs=hp_f, start=False, stop=True, skip_group_check=True)
                mm(pf6[:, 1, dt * Bc:(dt + 1) * Bc], lhsT=uf[:, 128:256],
                   rhs=hp_f, start=False, stop=True, skip_group_check=True)
                mm(sc[:, 0:Bc], lhsT=uf[:, 256:384],
                   rhs=hp_f, start=True, stop=True)
                mm(pf6[:, 3, dtb * Bc:(dtb + 1) * Bc], lhsT=ub[:, 0:128],
                   rhs=hp_b, start=False, stop=True, skip_group_check=True)
                mm(pf6[:, 4, dtb * Bc:(dtb + 1) * Bc], lhsT=ub[:, 128:256],
                   rhs=hp_b, start=False, stop=True, skip_group_check=True)
                mm(sc[:, Bc:2 * Bc], lhsT=ub[:, 256:384],
                   rhs=hp_b, start=True, stop=True)

                # zr layout: [z_f | r_f | z_b | r_b] (16 each)
                zr = pwp.tile([P, 4 * Bc], F32, tag="zr")
                zr4 = zr[:].rearrange("p (r q) -> p r q", r=4)
                nc.scalar.activation(zr4[:, 0:2, :], pf6[:, 0:2, dt * Bc:(dt + 1) * Bc],
                                     AF.Sigmoid)
                nc.scalar.activation(zr4[:, 2:4, :], pf6[:, 3:5, dtb * Bc:(dtb + 1) * Bc],
                                     AF.Sigmoid)

                tt = pwp.tile([P, 2 * Bc], F32, tag="tt")
                nc.vector.scalar_tensor_tensor(tt[:, 0:Bc], in0=sc[:, 0:Bc],
                                               scalar=brf[:, 0:1], in1=zr[:, Bc:2 * Bc],
                                               op0=ALU.add, op1=ALU.mult)
                nc.vector.scalar_tensor_tensor(tt[:, Bc:2 * Bc], in0=sc[:, Bc:2 * Bc],
                                               scalar=brb[:, 0:1], in1=zr[:, 3 * Bc:4 * Bc],
                                               op0=ALU.add, op1=ALU.mult)

                u = pwp.tile([P, 2 * Bc], F32, tag="u")
                nc.vector.tensor_tensor(u[:, 0:Bc], tt[:, 0:Bc],
                                        pf6[:, 2, dt * Bc:(dt + 1) * Bc], op=ALU.add)
                nc.vector.tensor_tensor(u[:, Bc:2 * Bc], tt[:, Bc:2 * Bc],
                                        pf6[:, 5, dtb * Bc:(dtb + 1) * Bc], op=ALU.add)

                hh = pwp.tile([P, 2 * Bc], F32, tag="hh")
                nc.scalar.activation(hh[:], u[:], AF.Tanh)
                hh2 = hh[:].rearrange("p (d b) -> p d b", d=2)

                dd = pwp.tile([P, 2 * Bc], F32, tag="dd")
                dd2 = dd[:].rearrange("p (d b) -> p d b", d=2)
                nc.vector.tensor_tensor(dd2, hstate3(s - 1), hh2, op=ALU.subtract)

                ee = pwp.tile([P, 2 * Bc], F32, tag="ee")
                ee2 = ee[:].rearrange("p (d b) -> p d b", d=2)
                z3 = zr[:].rearrange("p (d r q) -> p d r q", d=2, r=2)[:, :, 0, :]
                nc.vector.tensor_tensor(ee2, z3, dd2, op=ALU.mult)

                nc.vector.tensor_tensor(hstate3(s), hh2, ee2, op=ALU.add)

                # ---- output flush -------------------------------------
                if (s + 1) % FB == 0:
                    j = s // FB
                    jb = T // FB - 1 - j
                    for b in range(Bc):
                        tp = tpp.tile([P, P], F32, tag="tp")
                        tr(tp[0:FB, 0:P],
                           oa[:, b * T + j * FB: b * T + (j + 1) * FB], ident[:])
                        ob = pwp.tile([P, P], F32, tag="ob")
                        nc.scalar.copy(ob[0:FB, 0:P], tp[0:FB, 0:P])
                        nc.sync.dma_start(out_d[b, j * FB:(j + 1) * FB, 0:H],
                                          ob[0:FB, 0:P])
                        tp2 = tpp.tile([P, P], F32, tag="tp")
                        tr(tp2[0:FB, 0:P],
                           oa[:, NT + b * T + jb * FB: NT + b * T + (jb + 1) * FB],
                           ident[:])
                        ob2 = pwp.tile([P, P], F32, tag="ob")
                        nc.scalar.copy(ob2[0:FB, 0:P], tp2[0:FB, 0:P])
                        nc.sync.dma_start(out_d[b, jb * FB:(jb + 1) * FB, H:2 * H],
                                          ob2[0:FB, 0:P])

            # ---- final states -----------------------------------------
            stg_f = pwp.tile([P, Bc], F32, tag="stg")
            nc.vector.tensor_copy(stg_f[:], hstate_dir(T - 1, 0))
            nc.sync.dma_start(hf_d[:], stg_f[:])
            stg_b = pwp.tile([P, Bc], F32, tag="stg")
            nc.vector.tensor_copy(stg_b[:], hstate_dir(T - 1, 1))
            nc.sync.dma_start(hb_d[:], stg_b[:])

    nc.finalize()
    return nc


@functools.lru_cache(maxsize=2)
def _cached_program(T, Bc):
    return build_program(T, Bc)


def _host_inputs(encoder_input, state_fwd, state_back, emb, W_f, U_f, b_f,
                 W_b, U_b, b_b, T, Bc, n_cores):
    """Build per-core in_maps (plain numpy, layout prep only)."""
    ntile = T * Bc // P
    emb = np.ascontiguousarray(emb, dtype=np.float32)

    def w_aug(W, b2):
        bias = b2[0] + np.concatenate([b2[1, :256], np.zeros(128, np.float32)])
        return np.concatenate([W, bias[None, :].astype(np.float32)], axis=0)

    wf_a = w_aug(W_f, b_f)   # [301, 384]
    wb_a = w_aug(W_b, b_b)
    shared = {
        "emb": emb,
        "wf0": np.ascontiguousarray(wf_a[0:128]),
        "wf1": np.ascontiguousarray(wf_a[128:256]),
        "wf2": np.ascontiguousarray(wf_a[256:301]),
        "wb0": np.ascontiguousarray(wb_a[0:128]),
        "wb1": np.ascontiguousarray(wb_a[128:256]),
        "wb2": np.ascontiguousarray(wb_a[256:301]),
        "uf": np.ascontiguousarray(U_f, dtype=np.float32),
        "ub": np.ascontiguousarray(U_b, dtype=np.float32),
        "brf": np.ascontiguousarray(b_f[1, 256:384].reshape(H, 1)),
        "brb": np.ascontiguousarray(b_b[1, 256:384].reshape(H, 1)),
    }
    in_maps = []
    for c in range(n_cores):
        sl = slice(c * Bc, (c + 1) * Bc)
        enc = encoder_input[sl]                      # [Bc, T]
        idx_flat = np.ascontiguousarray(enc.T).reshape(-1)   # n = t*Bc + b
        idx_sb = np.ascontiguousarray(idx_flat.reshape(ntile, P).T,
                                      dtype=np.int32)        # [128, ntile]
        h0 = np.concatenate([state_fwd[sl].T, state_back[sl].T],
                            axis=1).astype(np.float32)       # [128, 2*Bc]
        in_maps.append(dict(shared, idx=idx_sb, h0=h0))
    return in_maps


def run_sharded(encoder_input, state_fwd, state_back, emb, W_f, U_f, b_f,
                W_b, U_b, b_b, T=None, Bc=None, n_cores=None, trace=False):
    B = encoder_input.shape[0]
    T = T or encoder_input.shape[1]
    n_cores = n_cores or N_CORES
    Bc = Bc or B // n_cores
    nc = _cached_program(T, Bc)
    in_maps = _host_inputs(encoder_input, state_fwd, state_back, emb,
                           W_f, U_f, b_f, W_b, U_b, b_b, T, Bc, n_cores)
    res = run_bass_kernel_spmd(nc, in_maps, core_ids=list(range(n_cores)),
                               trace=trace)
    outs = res.results
    enc_out = np.concatenate([o["out"] for o in outs], axis=0)
    h_f = np.concatenate([o["hf"].T for o in outs], axis=0)
    h_b = np.concatenate([o["hb"].T for o in outs], axis=0)
    return (enc_out, h_f, h_b), res


def kernel(encoder_input, state_fwd, state_back, emb, W_f, U_f, b_f,
           W_b, U_b, b_b):
    (enc_out, h_f, h_b), _ = run_sharded(
        np.asarray(encoder_input), np.asarray(state_fwd, dtype=np.float32),
        np.asarray(state_back, dtype=np.float32), np.asarray(emb, dtype=np.float32),
        np.asarray(W_f, dtype=np.float32), np.asarray(U_f, dtype=np.float32),
        np.asarray(b_f, dtype=np.float32), np.asarray(W_b, dtype=np.float32),
        np.asarray(U_b, dtype=np.float32), np.asarray(b_b, dtype=np.float32))
    return enc_out, h_f, h_b


# revision 7
# speedup vs baseline: 1.8147x; 1.8147x over previous
"""Bidirectional GRU encoder (Keras reset_after=True) on 8 Trainium2 NeuronCores.

Problem (hardcoded): B=128, T=512, V=32000, D=300, H=128, fp32.
  x = emb[encoder_input]                       # [B,T,300] gather
  out_f, h_f = GRU_fwd(x);  out_b, h_b = GRU_bwd(x reversed)
  return concat([out_f, out_b], -1), h_f, h_b

Sharding: data-parallel over batch, 16 rows per core; both directions run on
every core.  Inside a core everything is feature-major [H=128 partitions,
batch] so the recurrent state feeds the gate matmuls with no transposes.

Device-side plan per core (one Tile program, fully unrolled over T):
  - embedding rows are gathered 128 at a time with indirect DMA into
    [128, 304] row tiles, PE-transposed into x^T tiles [d_chunk, 128]
    (d chunks 128/128/45; the 45th row of chunk 2 is a constant 1 used to
    fold all additive biases into the input-projection matmul),
  - input projections x@W+b for both directions are computed chunk-by-chunk
    (8 timesteps at a time) straight into PSUM; the per-step recurrent
    matmuls U.T @ h accumulate into the same PSUM regions (z, r gates),
  - per step: sigmoid (z,r) -> (rec_h + b_rh) * r -> + x_h -> tanh ->
    h' = hh + z*(h - hh), written directly into the big SBUF output buffer
    which doubles as the recurrent state,
  - every 128 steps finished output columns are PE-transposed and DMA'd to
    DRAM as [b, t, h].
"""

import os
import sys
import functools

import numpy as np

for _p in ("/opt/trn_rl_repo",):
    if _p not in sys.path and os.path.isdir(_p):
        sys.path.insert(0, _p)

import concourse.bass as bass
import concourse.mybir as mybir
import concourse.tile as tile
from concourse import bacc
from concourse.bass import AP, IndirectOffsetOnAxis
from concourse.bass_utils import run_bass_kernel_spmd
from concourse.masks import make_identity

F32 = mybir.dt.float32
BF16 = mybir.dt.bfloat16
I32 = mybir.dt.int32
AF = mybir.ActivationFunctionType
ALU = mybir.AluOpType

# Problem constants
B_FULL, T_FULL, V, D, H = 128, 512, 32000, 300, 128
N_CORES = 8
P = 128  # partitions


def build_program(T: int, Bc: int):
    """Build the single-core Bass/Tile program (SPMD across cores)."""
    TC = P // Bc            # timesteps per psum chunk (8 for Bc=16)
    NT = T * Bc             # total (t, b) positions per direction
    ntile = NT // P         # 128-row tiles of gathered x
    RW = Bc * TC            # region width in psum chunk = 128
    FB = min(128, T)        # output flush block (timesteps)
    DCH = [(0, 128), (128, 128), (256, 45)]  # D-chunks incl. bias row
    assert T % TC == 0 and T % FB == 0 and NT % P == 0

    nc = bacc.Bacc("TRN2", target_bir_lowering=False, debug=False)

    # ---- DRAM tensors -------------------------------------------------
    idx_d = nc.dram_tensor("idx", [P, ntile], I32, kind="ExternalInput")
    emb_d = nc.dram_tensor("emb", [V, D], F32, kind="ExternalInput")
    w_d = {}
    for dname in ("f", "b"):
        for c, (k0, ks) in enumerate(DCH):
            w_d[dname, c] = nc.dram_tensor(f"w{dname}{c}", [ks, 384], BF16,
                                           kind="ExternalInput")
    uf_d = nc.dram_tensor("uf", [H, 384], BF16, kind="ExternalInput")
    ub_d = nc.dram_tensor("ub", [H, 384], BF16, kind="ExternalInput")
    brf_d = nc.dram_tensor("brf", [H, 1], F32, kind="ExternalInput")
    brb_d = nc.dram_tensor("brb", [H, 1], F32, kind="ExternalInput")
    h0_d = nc.dram_tensor("h0", [H, 2 * Bc], F32, kind="ExternalInput")
    h0b_d = nc.dram_tensor("h0b16", [H, 2 * Bc], BF16, kind="ExternalInput")

    out_d = nc.dram_tensor("out", [Bc, T, 2 * H], F32, kind="ExternalOutput")
    hf_d = nc.dram_tensor("hf", [H, Bc], F32, kind="ExternalOutput")
    hb_d = nc.dram_tensor("hb", [H, Bc], F32, kind="ExternalOutput")

    with tile.TileContext(nc) as tc:
        from contextlib import ExitStack
        with ExitStack() as ctx:
            cst = ctx.enter_context(tc.tile_pool(name="cst", bufs=1))
            xtp = ctx.enter_context(tc.tile_pool(name="xtp", bufs=ntile))
            oap = ctx.enter_context(tc.tile_pool(name="oap", bufs=1))
            xrw = ctx.enter_context(tc.tile_pool(name="xrw", bufs=3))
            pwp = ctx.enter_context(tc.tile_pool(name="pwp", bufs=3))
            chp = ctx.enter_context(tc.tile_pool(name="chp", bufs=2, space="PSUM"))
            scp = ctx.enter_context(tc.tile_pool(name="scp", bufs=2, space="PSUM"))
            tpp = ctx.enter_context(tc.tile_pool(name="tpp", bufs=2, space="PSUM"))

            # Bacc's compile passes split multi-sem waits into
            # EventSemaphore instructions (HW allows 1 wait per inst).
            def mm(out, lhsT, rhs, **kw):
                return nc.tensor.matmul(out, lhsT=lhsT, rhs=rhs, **kw)

            def tr(out, in_, identity):
                return nc.tensor.transpose(out, in_, identity)

            # ---- constants / weights into SBUF ------------------------
            ident = cst.tile([P, P], F32, tag="ident")
            make_identity(nc, ident[:])

            idx_sb = cst.tile([P, ntile], I32, tag="idx")
            nc.sync.dma_start(idx_sb[:], idx_d[:])

            w_sb = {}
            for dname in ("f", "b"):
                for c, (k0, ks) in enumerate(DCH):
                    t = cst.tile([ks, 384], BF16, tag=f"w{dname}{c}")
                    nc.sync.dma_start(t[:], w_d[dname, c][:])
                    w_sb[dname, c] = t
            uf = cst.tile([H, 384], BF16, tag="uf")
            nc.sync.dma_start(uf[:], uf_d[:])
            ub = cst.tile([H, 384], BF16, tag="ub")
            nc.sync.dma_start(ub[:], ub_d[:])
            brf = cst.tile([H, 1], F32, tag="brf")
            nc.sync.dma_start(brf[:], brf_d[:])
            brb = cst.tile([H, 1], F32, tag="brb")
            nc.sync.dma_start(brb[:], brb_d[:])
            h0sb = cst.tile([H, 2 * Bc], F32, tag="h0")
            nc.sync.dma_start(h0sb[:], h0_d[:])
            h0bf = cst.tile([H, 2 * Bc], BF16, tag="h0b16")
            nc.sync.dma_start(h0bf[:], h0b_d[:])
            hbfp = ctx.enter_context(tc.tile_pool(name="hbfp", bufs=2))

            # big persistent buffers
            out_all = oap.tile([P, 2 * NT], F32, tag="out_all")
            oa = out_all[:]
            oa_p = oa.ap[0]  # partition dim [stride, 128]

            def hstate3(s):
                """Combined state [128, 2, 16]: dir-f cols b*T+s, dir-b cols
                NT + b*T + (T-1-s).  Affine per fixed s."""
                if s < 0:
                    return h0sb[:].rearrange("p (d b) -> p d b", d=2)
                off_f = s
                off_b = NT + (T - 1 - s)
                return AP(oa.tensor, oa.offset + off_f,
                          [oa_p, [off_b - off_f, 2], [T, Bc]])

            def hstate_dir(s, d):
                if s < 0:
                    return h0sb[:, d * Bc:(d + 1) * Bc]
                off = s if d == 0 else NT + (T - 1 - s)
                return AP(oa.tensor, oa.offset + off, [oa_p, [T, Bc]])

            # ---- gather + transpose x into feature-major tiles ---------
            # x row n = t*Bc + b ; tile k covers n in [128k, 128k+128)
            xts = {0: [], 1: [], 2: []}
            order = []
            lo, hi = 0, ntile - 1
            while lo <= hi:
                order.append(lo)
                if hi != lo:
                    order.append(hi)
                lo += 1
                hi -= 1

            for k in order:
                xr = xrw.tile([P, 304], F32, tag="xr")
                nc.vector.memset(xr[:, 300:301], 1.0)
                nc.gpsimd.indirect_dma_start(
                    out=xr[:, 0:D], out_offset=None, in_=emb_d[:],
                    in_offset=IndirectOffsetOnAxis(ap=idx_sb[:, k:k + 1], axis=0))
                for c, (k0, ks) in enumerate(DCH):
                    tp = tpp.tile([P, P], F32, tag="tp")
                    tr(tp[0:ks, 0:P], xr[:, k0:k0 + ks], ident[:])
                    xt = xtp.tile([ks, P], BF16, tag=f"xt{c}")
                    nc.scalar.copy(xt[:], tp[0:ks, 0:P])
                    xts[c].append((k, xt))
            xt_of = {c: dict(xts[c]) for c in range(3)}

            # ---- psum chunk fill --------------------------------------
            # chunk regions (RW=128 cols each):
            #   0: z_f   1: r_f   2: xh_f   3: z_b   4: r_b   5: xh_b
            def fill_chunk(kc):
                # PSUM has_written semantics: a start=True matmul clears the
                # accumulate-bits of its ENTIRE bank.  So: exactly one
                # start=True per bank per chunk generation (the first matmul
                # touching that bank); everything else start=False —
                # unwritten-since-clear elements are overwritten, written
                # ones accumulate.  The per-step recurrent matmuls later
                # accumulate with start=False onto these regions.
                pf = chp.tile([P, 6 * RW], F32, tag="ch")
                pf6 = pf[:].rearrange("p (r q) -> p r q", r=6)
                for dbase, dname, ktile in ((0, "f", kc), (3, "b", ntile - 1 - kc)):
                    for g in range(3):
                        reg = dbase + g
                        for c, (k0, ks) in enumerate(DCH):
                            first_in_bank = (reg * RW) % 512 == 0 and c == 0
                            mm(pf6[:, reg, :],
                               lhsT=w_sb[dname, c][:, g * 128:(g + 1) * 128],
                               rhs=xt_of[c][ktile][:],
                               start=first_in_bank, stop=(c == 2),
                               skip_group_check=True)
                return pf

            # ---- recurrence -------------------------------------------
            pf = None
            pf6 = None
            for s in range(T):
                kc, dt = divmod(s, TC)
                dtb = TC - 1 - dt
                if dt == 0:
                    pf = fill_chunk(kc)
                    pf6 = pf[:].rearrange("p (r q) -> p r q", r=6)

                hp = h0bf[:] if s == 0 else h_bf[:]
                hp_f = hp[:, 0:Bc]
                hp_b = hp[:, Bc:2 * Bc]
                sc = scp.tile([P, 2 * Bc], F32, tag="sc")

                mm(pf6[:, 0, dt * Bc:(dt + 1) * Bc], lhsT=uf[:, 0:128],
                   rhs=hp_f, start=False, stop=True, skip_group_check=True)
                mm(pf6[:, 1, dt * Bc:(dt + 1) * Bc], lhsT=uf[:, 128:256],
                   rhs=hp_f, start=False, stop=True, skip_group_check=True)
                mm(sc[:, 0:Bc], lhsT=uf[:, 256:384],
                   rhs=hp_f, start=True, stop=True)
                mm(pf6[:, 3, dtb * Bc:(dtb + 1) * Bc], lhsT=ub[:, 0:128],
                   rhs=hp_b, start=False, stop=True, skip_group_check=True)
                mm(pf6[:, 4, dtb * Bc:(dtb + 1) * Bc], lhsT=ub[:, 128:256],
                   rhs=hp_b, start=False, stop=True, skip_group_check=True)
                mm(sc[:, Bc:2 * Bc], lhsT=ub[:, 256:384],
                   rhs=hp_b, start=True, stop=True)

                # zr layout: [z_f | r_f | z_b | r_b] (16 each)
                zr = pwp.tile([P, 4 * Bc], F32, tag="zr")
                zr4 = zr[:].rearrange("p (r q) -> p r q", r=4)
                nc.scalar.activation(zr4[:, 0:2, :], pf6[:, 0:2, dt * Bc:(dt + 1) * Bc],
                                     AF.Sigmoid)
                nc.scalar.activation(zr4[:, 2:4, :], pf6[:, 3:5, dtb * Bc:(dtb + 1) * Bc],
                                     AF.Sigmoid)

                tt = pwp.tile([P, 2 * Bc], F32, tag="tt")
                nc.vector.scalar_tensor_tensor(tt[:, 0:Bc], in0=sc[:, 0:Bc],
                                               scalar=brf[:, 0:1], in1=zr[:, Bc:2 * Bc],
                                               op0=ALU.add, op1=ALU.mult)
                nc.vector.scalar_tensor_tensor(tt[:, Bc:2 * Bc], in0=sc[:, Bc:2 * Bc],
                                               scalar=brb[:, 0:1], in1=zr[:, 3 * Bc:4 * Bc],
                                               op0=ALU.add, op1=ALU.mult)

                u = pwp.tile([P, 2 * Bc], F32, tag="u")
                nc.vector.tensor_tensor(u[:, 0:Bc], tt[:, 0:Bc],
                                        pf6[:, 2, dt * Bc:(dt + 1) * Bc], op=ALU.add)
                nc.vector.tensor_tensor(u[:, Bc:2 * Bc], tt[:, Bc:2 * Bc],
                                        pf6[:, 5, dtb * Bc:(dtb + 1) * Bc], op=ALU.add)

                hh = pwp.tile([P, 2 * Bc], F32, tag="hh")
                nc.scalar.activation(hh[:], u[:], AF.Tanh)
                hh2 = hh[:].rearrange("p (d b) -> p d b", d=2)

                dd = pwp.tile([P, 2 * Bc], F32, tag="dd")
                dd2 = dd[:].rearrange("p (d b) -> p d b", d=2)
                nc.vector.tensor_tensor(dd2, hstate3(s - 1), hh2, op=ALU.subtract)

                ee = pwp.tile([P, 2 * Bc], F32, tag="ee")
                ee2 = ee[:].rearrange("p (d b) -> p d b", d=2)
                z3 = zr[:].rearrange("p (d r q) -> p d r q", d=2, r=2)[:, :, 0, :]
                nc.vector.tensor_tensor(ee2, z3, dd2, op=ALU.mult)

                h_bf = hbfp.tile([P, 2 * Bc], BF16, tag="hbf")
                nc.vector.tensor_tensor(h_bf[:], hh[:], ee[:], op=ALU.add)
                nc.vector.tensor_tensor(hstate3(s), hh2, ee2, op=ALU.add)

                # ---- output flush -------------------------------------
                if (s + 1) % FB == 0:
                    j = s // FB
                    jb = T // FB - 1 - j
                    for b in range(Bc):
                        tp = tpp.tile([P, P], F32, tag="tp")
                        tr(tp[0:FB, 0:P],
                           oa[:, b * T + j * FB: b * T + (j + 1) * FB], ident[:])
                        ob = pwp.tile([P, P], F32, tag="ob")
                        nc.scalar.copy(ob[0:FB, 0:P], tp[0:FB, 0:P])
                        nc.sync.dma_start(out_d[b, j * FB:(j + 1) * FB, 0:H],
                                          ob[0:FB, 0:P])
                        tp2 = tpp.tile([P, P], F32, tag="tp")
                        tr(tp2[0:FB, 0:P],
                           oa[:, NT + b * T + jb * FB: NT + b * T + (jb + 1) * FB],
                           ident[:])
                        ob2 = pwp.tile([P, P], F32, tag="ob")
                        nc.scalar.copy(ob2[0:FB, 0:P], tp2[0:FB, 0:P])
                        nc.sync.dma_start(out_d[b, jb * FB:(jb + 1) * FB, H:2 * H],
                                          ob2[0:FB, 0:P])

            # ---- final states -----------------------------------------
            stg_f = pwp.tile([P, Bc], F32, tag="stg")
            nc.vector.tensor_copy(stg_f[:], hstate_dir(T - 1, 0))
            nc.sync.dma_start(hf_d[:], stg_f[:])
            stg_b = pwp.tile([P, Bc], F32, tag="stg")
            nc.vector.tensor_copy(stg_b[:], hstate_dir(T - 1, 1))
            nc.sync.dma_start(hb_d[:], stg_b[:])

    nc.finalize()
    return nc


@functools.lru_cache(maxsize=2)
def _cached_program(T, Bc):
    return build_program(T, Bc)


def _host_inputs(encoder_input, state_fwd, state_back, emb, W_f, U_f, b_f,
                 W_b, U_b, b_b, T, Bc, n_cores):
    """Build per-core in_maps (plain numpy, layout prep only)."""
    ntile = T * Bc // P
    emb = np.ascontiguousarray(emb, dtype=np.float32)

    def w_aug(W, b2):
        bias = b2[0] + np.concatenate([b2[1, :256], np.zeros(128, np.float32)])
        return np.concatenate([W, bias[None, :].astype(np.float32)], axis=0)

    wf_a = w_aug(W_f, b_f)   # [301, 384]
    wb_a = w_aug(W_b, b_b)
    import ml_dtypes
    bf16 = ml_dtypes.bfloat16
    shared = {
        "emb": emb,
        "wf0": np.ascontiguousarray(wf_a[0:128]).astype(bf16),
        "wf1": np.ascontiguousarray(wf_a[128:256]).astype(bf16),
        "wf2": np.ascontiguousarray(wf_a[256:301]).astype(bf16),
        "wb0": np.ascontiguousarray(wb_a[0:128]).astype(bf16),
        "wb1": np.ascontiguousarray(wb_a[128:256]).astype(bf16),
        "wb2": np.ascontiguousarray(wb_a[256:301]).astype(bf16),
        "uf": np.ascontiguousarray(U_f, dtype=np.float32).astype(bf16),
        "ub": np.ascontiguousarray(U_b, dtype=np.float32).astype(bf16),
        "brf": np.ascontiguousarray(b_f[1, 256:384].reshape(H, 1)),
        "brb": np.ascontiguousarray(b_b[1, 256:384].reshape(H, 1)),
    }
    in_maps = []
    for c in range(n_cores):
        sl = slice(c * Bc, (c + 1) * Bc)
        enc = encoder_input[sl]                      # [Bc, T]
        idx_flat = np.ascontiguousarray(enc.T).reshape(-1)   # n = t*Bc + b
        idx_sb = np.ascontiguousarray(idx_flat.reshape(ntile, P).T,
                                      dtype=np.int32)        # [128, ntile]
        h0 = np.concatenate([state_fwd[sl].T, state_back[sl].T],
                            axis=1).astype(np.float32)       # [128, 2*Bc]
        in_maps.append(dict(shared, idx=idx_sb, h0=h0, h0b16=h0.astype(bf16)))
    return in_maps


def run_sharded(encoder_input, state_fwd, state_back, emb, W_f, U_f, b_f,
                W_b, U_b, b_b, T=None, Bc=None, n_cores=None, trace=False):
    B = encoder_input.shape[0]
    T = T or encoder_input.shape[1]
    n_cores = n_cores or N_CORES
    Bc = Bc or B // n_cores
    nc = _cached_program(T, Bc)
    in_maps = _host_inputs(encoder_input, state_fwd, state_back, emb,
                           W_f, U_f, b_f, W_b, U_b, b_b, T, Bc, n_cores)
    res = run_bass_kernel_spmd(nc, in_maps, core_ids=list(range(n_cores)),
                               trace=trace)
    outs = res.results
    enc_out = np.concatenate([o["out"] for o in outs], axis=0)
    h_f = np.concatenate([o["hf"].T for o in outs], axis=0)
    h_b = np.concatenate([o["hb"].T for o in outs], axis=0)
    return (enc_out, h_f, h_b), res


def kernel(encoder_input, state_fwd, state_back, emb, W_f, U_f, b_f,
           W_b, U_b, b_b):
    (enc_out, h_f, h_b), _ = run_sharded(
        np.asarray(encoder_input), np.asarray(state_fwd, dtype=np.float32),
        np.asarray(state_back, dtype=np.float32), np.asarray(emb, dtype=np.float32),
        np.asarray(W_f, dtype=np.float32), np.asarray(U_f, dtype=np.float32),
        np.asarray(b_f, dtype=np.float32), np.asarray(W_b, dtype=np.float32),
        np.asarray(U_b, dtype=np.float32), np.asarray(b_b, dtype=np.float32))
    return enc_out, h_f, h_b


# revision 8
# speedup vs baseline: 1.9436x; 1.0711x over previous
"""Bidirectional GRU encoder (Keras reset_after=True) on 8 Trainium2 NeuronCores.

Problem (hardcoded): B=128, T=512, V=32000, D=300, H=128, fp32.
  x = emb[encoder_input]                       # [B,T,300] gather
  out_f, h_f = GRU_fwd(x);  out_b, h_b = GRU_bwd(x reversed)
  return concat([out_f, out_b], -1), h_f, h_b

Sharding: data-parallel over batch, 16 rows per core; both directions run on
every core.  Inside a core everything is feature-major [H=128 partitions,
batch] so the recurrent state feeds the gate matmuls with no transposes.

Device-side plan per core (one Tile program, fully unrolled over T):
  - embedding rows are gathered 128 at a time with indirect DMA into
    [128, 304] row tiles, PE-transposed into x^T tiles [d_chunk, 128]
    (d chunks 128/128/45; the 45th row of chunk 2 is a constant 1 used to
    fold all additive biases into the input-projection matmul),
  - input projections x@W+b for both directions are computed chunk-by-chunk
    (8 timesteps at a time) straight into PSUM; the per-step recurrent
    matmuls U.T @ h accumulate into the same PSUM regions (z, r gates),
  - per step: sigmoid (z,r) -> (rec_h + b_rh) * r -> + x_h -> tanh ->
    h' = hh + z*(h - hh), written directly into the big SBUF output buffer
    which doubles as the recurrent state,
  - every 128 steps finished output columns are PE-transposed and DMA'd to
    DRAM as [b, t, h].
"""

import os
import sys
import functools

import numpy as np

for _p in ("/opt/trn_rl_repo",):
    if _p not in sys.path and os.path.isdir(_p):
        sys.path.insert(0, _p)

import concourse.bass as bass
import concourse.mybir as mybir
import concourse.tile as tile
from concourse import bacc
from concourse.bass import AP, IndirectOffsetOnAxis
from concourse.bass_utils import run_bass_kernel_spmd
from concourse.masks import make_identity

F32 = mybir.dt.float32
BF16 = mybir.dt.bfloat16
I32 = mybir.dt.int32
AF = mybir.ActivationFunctionType
ALU = mybir.AluOpType

# Problem constants
B_FULL, T_FULL, V, D, H = 128, 512, 32000, 300, 128
N_CORES = 8
P = 128  # partitions


def build_program(T: int, Bc: int):
    """Build the single-core Bass/Tile program (SPMD across cores)."""
    TC = P // Bc            # timesteps per psum chunk (8 for Bc=16)
    NT = T * Bc             # total (t, b) positions per direction
    ntile = NT // P         # 128-row tiles of gathered x
    RW = Bc * TC            # region width in psum chunk = 128
    FB = min(128, T)        # output flush block (timesteps)
    DCH = [(0, 128), (128, 128), (256, 45)]  # D-chunks incl. bias row
    assert T % TC == 0 and T % FB == 0 and NT % P == 0

    nc = bacc.Bacc("TRN2", target_bir_lowering=False, debug=False)

    # ---- DRAM tensors -------------------------------------------------
    idx_d = nc.dram_tensor("idx", [P, ntile], I32, kind="ExternalInput")
    emb_d = nc.dram_tensor("emb", [V, D], F32, kind="ExternalInput")
    w_d = {}
    for dname in ("f", "b"):
        for c, (k0, ks) in enumerate(DCH):
            w_d[dname, c] = nc.dram_tensor(f"w{dname}{c}", [ks, 384], BF16,
                                           kind="ExternalInput")
    uf_d = nc.dram_tensor("uf", [H, 384], BF16, kind="ExternalInput")
    ub_d = nc.dram_tensor("ub", [H, 384], BF16, kind="ExternalInput")
    brf_d = nc.dram_tensor("brf", [H, 1], F32, kind="ExternalInput")
    brb_d = nc.dram_tensor("brb", [H, 1], F32, kind="ExternalInput")
    h0_d = nc.dram_tensor("h0", [H, 2 * Bc], F32, kind="ExternalInput")
    h0b_d = nc.dram_tensor("h0b16", [H, 2 * Bc], BF16, kind="ExternalInput")

    out_d = nc.dram_tensor("out", [Bc, T, 2 * H], F32, kind="ExternalOutput")
    hf_d = nc.dram_tensor("hf", [H, Bc], F32, kind="ExternalOutput")
    hb_d = nc.dram_tensor("hb", [H, Bc], F32, kind="ExternalOutput")

    with tile.TileContext(nc) as tc:
        from contextlib import ExitStack
        with ExitStack() as ctx:
            cst = ctx.enter_context(tc.tile_pool(name="cst", bufs=1))
            xtp = ctx.enter_context(tc.tile_pool(name="xtp", bufs=ntile))
            oap = ctx.enter_context(tc.tile_pool(name="oap", bufs=1))
            xrw = ctx.enter_context(tc.tile_pool(name="xrw", bufs=3))
            pwp = ctx.enter_context(tc.tile_pool(name="pwp", bufs=3))
            chp = ctx.enter_context(tc.tile_pool(name="chp", bufs=2, space="PSUM"))
            scp = ctx.enter_context(tc.tile_pool(name="scp", bufs=2, space="PSUM"))
            tpp = ctx.enter_context(tc.tile_pool(name="tpp", bufs=2, space="PSUM"))

            # Bacc's compile passes split multi-sem waits into
            # EventSemaphore instructions (HW allows 1 wait per inst).
            def mm(out, lhsT, rhs, **kw):
                return nc.tensor.matmul(out, lhsT=lhsT, rhs=rhs, **kw)

            def tr(out, in_, identity):
                return nc.tensor.transpose(out, in_, identity)

            # ---- constants / weights into SBUF ------------------------
            ident = cst.tile([P, P], F32, tag="ident")
            make_identity(nc, ident[:])

            idx_sb = cst.tile([P, ntile], I32, tag="idx")
            nc.sync.dma_start(idx_sb[:], idx_d[:])

            w_sb = {}
            for dname in ("f", "b"):
                for c, (k0, ks) in enumerate(DCH):
                    t = cst.tile([ks, 384], BF16, tag=f"w{dname}{c}")
                    nc.sync.dma_start(t[:], w_d[dname, c][:])
                    w_sb[dname, c] = t
            uf = cst.tile([H, 384], BF16, tag="uf")
            nc.sync.dma_start(uf[:], uf_d[:])
            ub = cst.tile([H, 384], BF16, tag="ub")
            nc.sync.dma_start(ub[:], ub_d[:])
            brf = cst.tile([H, 1], F32, tag="brf")
            nc.sync.dma_start(brf[:], brf_d[:])
            brb = cst.tile([H, 1], F32, tag="brb")
            nc.sync.dma_start(brb[:], brb_d[:])
            h0sb = cst.tile([H, 2 * Bc], F32, tag="h0")
            nc.sync.dma_start(h0sb[:], h0_d[:])
            h0bf = cst.tile([H, 2 * Bc], BF16, tag="h0b16")
            nc.sync.dma_start(h0bf[:], h0b_d[:])
            hbfp = ctx.enter_context(tc.tile_pool(name="hbfp", bufs=2))

            # HAM warmup: ~5us of dense matmuls so the PE clock-gate
            # opens (K=8/8); recurrence-phase gaps are ~1us < the ~3.4us
            # MID window, so it stays warm afterwards.
            wup = chp.tile([P, 6 * RW], F32, tag="ch")
            for i in range(12):
                mm(wup[:, 0:384], lhsT=uf[:, 0:128], rhs=uf[:],
                   start=True, stop=True)

            # big persistent buffers
            out_all = oap.tile([P, 2 * NT], F32, tag="out_all")
            oa = out_all[:]
            oa_p = oa.ap[0]  # partition dim [stride, 128]

            def hstate3(s):
                """Combined state [128, 2, 16]: dir-f cols b*T+s, dir-b cols
                NT + b*T + (T-1-s).  Affine per fixed s."""
                if s < 0:
                    return h0sb[:].rearrange("p (d b) -> p d b", d=2)
                off_f = s
                off_b = NT + (T - 1 - s)
                return AP(oa.tensor, oa.offset + off_f,
                          [oa_p, [off_b - off_f, 2], [T, Bc]])

            def hstate_dir(s, d):
                if s < 0:
                    return h0sb[:, d * Bc:(d + 1) * Bc]
                off = s if d == 0 else NT + (T - 1 - s)
                return AP(oa.tensor, oa.offset + off, [oa_p, [T, Bc]])

            # ---- gather + transpose x into feature-major tiles ---------
            # x row n = t*Bc + b ; tile k covers n in [128k, 128k+128)
            xts = {0: [], 1: [], 2: []}
            order = []
            lo, hi = 0, ntile - 1
            while lo <= hi:
                order.append(lo)
                if hi != lo:
                    order.append(hi)
                lo += 1
                hi -= 1

            for k in order:
                xr = xrw.tile([P, 304], F32, tag="xr")
                nc.vector.memset(xr[:, 300:301], 1.0)
                nc.gpsimd.indirect_dma_start(
                    out=xr[:, 0:D], out_offset=None, in_=emb_d[:],
                    in_offset=IndirectOffsetOnAxis(ap=idx_sb[:, k:k + 1], axis=0))
                for c, (k0, ks) in enumerate(DCH):
                    tp = tpp.tile([P, P], F32, tag="tp")
                    tr(tp[0:ks, 0:P], xr[:, k0:k0 + ks], ident[:])
                    xt = xtp.tile([ks, P], BF16, tag=f"xt{c}")
                    nc.scalar.copy(xt[:], tp[0:ks, 0:P])
                    xts[c].append((k, xt))
            xt_of = {c: dict(xts[c]) for c in range(3)}

            # ---- psum chunk fill --------------------------------------
            # chunk regions (RW=128 cols each):
            #   0: z_f   1: r_f   2: xh_f   3: z_b   4: r_b   5: xh_b
            def fill_chunk(kc):
                # PSUM has_written semantics: a start=True matmul clears the
                # accumulate-bits of its ENTIRE bank.  So: exactly one
                # start=True per bank per chunk generation (the first matmul
                # touching that bank); everything else start=False —
                # unwritten-since-clear elements are overwritten, written
                # ones accumulate.  The per-step recurrent matmuls later
                # accumulate with start=False onto these regions.
                pf = chp.tile([P, 6 * RW], F32, tag="ch")
                pf6 = pf[:].rearrange("p (r q) -> p r q", r=6)
                for dbase, dname, ktile in ((0, "f", kc), (3, "b", ntile - 1 - kc)):
                    for g in range(3):
                        reg = dbase + g
                        for c, (k0, ks) in enumerate(DCH):
                            first_in_bank = (reg * RW) % 512 == 0 and c == 0
                            mm(pf6[:, reg, :],
                               lhsT=w_sb[dname, c][:, g * 128:(g + 1) * 128],
                               rhs=xt_of[c][ktile][:],
                               start=first_in_bank, stop=(c == 2),
                               skip_group_check=True)
                return pf

            # ---- recurrence -------------------------------------------
            pf = None
            pf6 = None
            for s in range(T):
                kc, dt = divmod(s, TC)
                dtb = TC - 1 - dt
                if dt == 0:
                    pf = fill_chunk(kc)
                    pf6 = pf[:].rearrange("p (r q) -> p r q", r=6)

                hp = h0bf[:] if s == 0 else h_bf[:]
                hp_f = hp[:, 0:Bc]
                hp_b = hp[:, Bc:2 * Bc]
                sc = scp.tile([P, 2 * Bc], F32, tag="sc")

                mm(pf6[:, 0, dt * Bc:(dt + 1) * Bc], lhsT=uf[:, 0:128],
                   rhs=hp_f, start=False, stop=True, skip_group_check=True)
                mm(pf6[:, 1, dt * Bc:(dt + 1) * Bc], lhsT=uf[:, 128:256],
                   rhs=hp_f, start=False, stop=True, skip_group_check=True)
                mm(sc[:, 0:Bc], lhsT=uf[:, 256:384],
                   rhs=hp_f, start=True, stop=True)
                mm(pf6[:, 3, dtb * Bc:(dtb + 1) * Bc], lhsT=ub[:, 0:128],
                   rhs=hp_b, start=False, stop=True, skip_group_check=True)
                mm(pf6[:, 4, dtb * Bc:(dtb + 1) * Bc], lhsT=ub[:, 128:256],
                   rhs=hp_b, start=False, stop=True, skip_group_check=True)
                mm(sc[:, Bc:2 * Bc], lhsT=ub[:, 256:384],
                   rhs=hp_b, start=True, stop=True)

                # zr layout: [z_f | r_f | z_b | r_b] (16 each)
                zr = pwp.tile([P, 4 * Bc], F32, tag="zr")
                zr4 = zr[:].rearrange("p (r q) -> p r q", r=4)
                nc.scalar.activation(zr4[:, 0:2, :], pf6[:, 0:2, dt * Bc:(dt + 1) * Bc],
                                     AF.Sigmoid)
                nc.scalar.activation(zr4[:, 2:4, :], pf6[:, 3:5, dtb * Bc:(dtb + 1) * Bc],
                                     AF.Sigmoid)

                tt = pwp.tile([P, 2 * Bc], F32, tag="tt")
                nc.vector.scalar_tensor_tensor(tt[:, 0:Bc], in0=sc[:, 0:Bc],
                                               scalar=brf[:, 0:1], in1=zr[:, Bc:2 * Bc],
                                               op0=ALU.add, op1=ALU.mult)
                nc.vector.scalar_tensor_tensor(tt[:, Bc:2 * Bc], in0=sc[:, Bc:2 * Bc],
                                               scalar=brb[:, 0:1], in1=zr[:, 3 * Bc:4 * Bc],
                                               op0=ALU.add, op1=ALU.mult)

                u = pwp.tile([P, 2 * Bc], F32, tag="u")
                nc.vector.tensor_tensor(u[:, 0:Bc], tt[:, 0:Bc],
                                        pf6[:, 2, dt * Bc:(dt + 1) * Bc], op=ALU.add)
                nc.vector.tensor_tensor(u[:, Bc:2 * Bc], tt[:, Bc:2 * Bc],
                                        pf6[:, 5, dtb * Bc:(dtb + 1) * Bc], op=ALU.add)

                hh = pwp.tile([P, 2 * Bc], F32, tag="hh")
                nc.scalar.activation(hh[:], u[:], AF.Tanh)

                # h' = z*h + (1-z)*hh ; w=1-z and zh=z*h run in the tanh
                # shadow so only v and h' sit on the serial chain.
                z3 = zr[:].rearrange("p (d r q) -> p d r q", d=2, r=2)[:, :, 0, :]
                w = pwp.tile([P, 2 * Bc], F32, tag="w")
                w2 = w[:].rearrange("p (d b) -> p d b", d=2)
                nc.vector.tensor_scalar(w2, z3, -1.0, 1.0, ALU.mult, ALU.add)
                zh = pwp.tile([P, 2 * Bc], F32, tag="zh")
                zh2 = zh[:].rearrange("p (d b) -> p d b", d=2)
                nc.vector.tensor_tensor(zh2, z3, hstate3(s - 1), op=ALU.mult)

                v = pwp.tile([P, 2 * Bc], F32, tag="v")
                nc.vector.tensor_tensor(v[:], w[:], hh[:], op=ALU.mult)
                h_bf = hbfp.tile([P, 2 * Bc], BF16, tag="hbf")
                nc.vector.tensor_tensor(h_bf[:], v[:], zh[:], op=ALU.add)
                nc.vector.tensor_tensor(hstate3(s), v[:].rearrange("p (d b) -> p d b", d=2),
                                        zh2, op=ALU.add)

                # ---- output flush -------------------------------------
                if (s + 1) % FB == 0:
                    j = s // FB
                    jb = T // FB - 1 - j
                    for b in range(Bc):
                        tp = tpp.tile([P, P], F32, tag="tp")
                        tr(tp[0:FB, 0:P],
                           oa[:, b * T + j * FB: b * T + (j + 1) * FB], ident[:])
                        ob = pwp.tile([P, P], F32, tag="ob")
                        if b % 2 == 0:
                            nc.scalar.copy(ob[0:FB, 0:P], tp[0:FB, 0:P])
                        else:
                            nc.vector.tensor_copy(ob[0:FB, 0:P], tp[0:FB, 0:P])
                        nc.sync.dma_start(out_d[b, j * FB:(j + 1) * FB, 0:H],
                                          ob[0:FB, 0:P])
                        tp2 = tpp.tile([P, P], F32, tag="tp")
                        tr(tp2[0:FB, 0:P],
                           oa[:, NT + b * T + jb * FB: NT + b * T + (jb + 1) * FB],
                           ident[:])
                        ob2 = pwp.tile([P, P], F32, tag="ob")
                        if b % 2 == 1:
                            nc.scalar.copy(ob2[0:FB, 0:P], tp2[0:FB, 0:P])
                        else:
                            nc.vector.tensor_copy(ob2[0:FB, 0:P], tp2[0:FB, 0:P])
                        nc.sync.dma_start(out_d[b, jb * FB:(jb + 1) * FB, H:2 * H],
                                          ob2[0:FB, 0:P])

            # ---- final states -----------------------------------------
            stg_f = pwp.tile([P, Bc], F32, tag="stg")
            nc.vector.tensor_copy(stg_f[:], hstate_dir(T - 1, 0))
            nc.sync.dma_start(hf_d[:], stg_f[:])
            stg_b = pwp.tile([P, Bc], F32, tag="stg")
            nc.vector.tensor_copy(stg_b[:], hstate_dir(T - 1, 1))
            nc.sync.dma_start(hb_d[:], stg_b[:])

    nc.finalize()
    return nc


@functools.lru_cache(maxsize=2)
def _cached_program(T, Bc):
    return build_program(T, Bc)


def _host_inputs(encoder_input, state_fwd, state_back, emb, W_f, U_f, b_f,
                 W_b, U_b, b_b, T, Bc, n_cores):
    """Build per-core in_maps (plain numpy, layout prep only)."""
    ntile = T * Bc // P
    emb = np.ascontiguousarray(emb, dtype=np.float32)

    def w_aug(W, b2):
        bias = b2[0] + np.concatenate([b2[1, :256], np.zeros(128, np.float32)])
        return np.concatenate([W, bias[None, :].astype(np.float32)], axis=0)

    wf_a = w_aug(W_f, b_f)   # [301, 384]
    wb_a = w_aug(W_b, b_b)
    import ml_dtypes
    bf16 = ml_dtypes.bfloat16
    shared = {
        "emb": emb,
        "wf0": np.ascontiguousarray(wf_a[0:128]).astype(bf16),
        "wf1": np.ascontiguousarray(wf_a[128:256]).astype(bf16),
        "wf2": np.ascontiguousarray(wf_a[256:301]).astype(bf16),
        "wb0": np.ascontiguousarray(wb_a[0:128]).astype(bf16),
        "wb1": np.ascontiguousarray(wb_a[128:256]).astype(bf16),
        "wb2": np.ascontiguousarray(wb_a[256:301]).astype(bf16),
        "uf": np.ascontiguousarray(U_f, dtype=np.float32).astype(bf16),
        "ub": np.ascontiguousarray(U_b, dtype=np.float32).astype(bf16),
        "brf": np.ascontiguousarray(b_f[1, 256:384].reshape(H, 1)),
        "brb": np.ascontiguousarray(b_b[1, 256:384].reshape(H, 1)),
    }
    in_maps = []
    for c in range(n_cores):
        sl = slice(c * Bc, (c + 1) * Bc)
        enc = encoder_input[sl]                      # [Bc, T]
        idx_flat = np.ascontiguousarray(enc.T).reshape(-1)   # n = t*Bc + b
        idx_sb = np.ascontiguousarray(idx_flat.reshape(ntile, P).T,
                                      dtype=np.int32)        # [128, ntile]
        h0 = np.concatenate([state_fwd[sl].T, state_back[sl].T],
                            axis=1).astype(np.float32)       # [128, 2*Bc]
        in_maps.append(dict(shared, idx=idx_sb, h0=h0, h0b16=h0.astype(bf16)))
    return in_maps


def run_sharded(encoder_input, state_fwd, state_back, emb, W_f, U_f, b_f,
                W_b, U_b, b_b, T=None, Bc=None, n_cores=None, trace=False):
    B = encoder_input.shape[0]
    T = T or encoder_input.shape[1]
    n_cores = n_cores or N_CORES
    Bc = Bc or B // n_cores
    nc = _cached_program(T, Bc)
    in_maps = _host_inputs(encoder_input, state_fwd, state_back, emb,
                           W_f, U_f, b_f, W_b, U_b, b_b, T, Bc, n_cores)
    res = run_bass_kernel_spmd(nc, in_maps, core_ids=list(range(n_cores)),
                               trace=trace)
    outs = res.results
    enc_out = np.concatenate([o["out"] for o in outs], axis=0)
    h_f = np.concatenate([o["hf"].T for o in outs], axis=0)
    h_b = np.concatenate([o["hb"].T for o in outs], axis=0)
    return (enc_out, h_f, h_b), res


def kernel(encoder_input, state_fwd, state_back, emb, W_f, U_f, b_f,
           W_b, U_b, b_b):
    (enc_out, h_f, h_b), _ = run_sharded(
        np.asarray(encoder_input), np.asarray(state_fwd, dtype=np.float32),
        np.asarray(state_back, dtype=np.float32), np.asarray(emb, dtype=np.float32),
        np.asarray(W_f, dtype=np.float32), np.asarray(U_f, dtype=np.float32),
        np.asarray(b_f, dtype=np.float32), np.asarray(W_b, dtype=np.float32),
        np.asarray(U_b, dtype=np.float32), np.asarray(b_b, dtype=np.float32))
    return enc_out, h_f, h_b


# revision 11
# speedup vs baseline: 2.0424x; 1.0508x over previous
"""Bidirectional GRU encoder (Keras reset_after=True) on 8 Trainium2 NeuronCores.

Problem (hardcoded): B=128, T=512, V=32000, D=300, H=128, fp32.
  x = emb[encoder_input]                       # [B,T,300] gather
  out_f, h_f = GRU_fwd(x);  out_b, h_b = GRU_bwd(x reversed)
  return concat([out_f, out_b], -1), h_f, h_b

Sharding: data-parallel over batch, 16 rows per core; both directions run on
every core.  Inside a core everything is feature-major [H=128 partitions,
batch] so the recurrent state feeds the gate matmuls with no transposes.

Device-side plan per core (one Tile program, fully unrolled over T):
  - embedding rows are gathered 128 at a time with indirect DMA into
    [128, 304] row tiles, PE-transposed into x^T tiles [d_chunk, 128]
    (d chunks 128/128/45; the 45th row of chunk 2 is a constant 1 used to
    fold all additive biases into the input-projection matmul),
  - input projections x@W+b for both directions are computed chunk-by-chunk
    (8 timesteps at a time) straight into PSUM; the per-step recurrent
    matmuls U.T @ h accumulate into the same PSUM regions (z, r gates),
  - per step: sigmoid (z,r) -> (rec_h + b_rh) * r -> + x_h -> tanh ->
    h' = hh + z*(h - hh), written directly into the big SBUF output buffer
    which doubles as the recurrent state,
  - every 128 steps finished output columns are PE-transposed and DMA'd to
    DRAM as [b, t, h].
"""

import os
import sys
import functools

import numpy as np

for _p in ("/opt/trn_rl_repo",):
    if _p not in sys.path and os.path.isdir(_p):
        sys.path.insert(0, _p)

import concourse.bass as bass
import concourse.mybir as mybir
import concourse.tile as tile
from concourse import bacc
from concourse.bass import AP, IndirectOffsetOnAxis
from concourse.bass_utils import run_bass_kernel_spmd
from concourse.masks import make_identity

F32 = mybir.dt.float32
BF16 = mybir.dt.bfloat16
I32 = mybir.dt.int32
AF = mybir.ActivationFunctionType
ALU = mybir.AluOpType

# Problem constants
B_FULL, T_FULL, V, D, H = 128, 512, 32000, 300, 128
N_CORES = 8
P = 128  # partitions


def build_program(T: int, Bc: int):
    """Build the single-core Bass/Tile program (SPMD across cores)."""
    TC = min(16, T)         # timesteps per psum chunk
    NT = T * Bc             # total (t, b) positions per direction
    ntile = NT // P         # 128-row tiles of gathered x
    RW = Bc * TC            # region width in psum chunk (256)
    XW = RW // P            # 128-col x tiles per chunk (2)
    nch = T // TC           # number of chunks
    FB = min(128, T)        # output flush block (timesteps)
    DCH = [(0, 128), (128, 128), (256, 45)]  # D-chunks incl. bias row
    assert T % TC == 0 and T % FB == 0 and NT % P == 0 and ntile % XW == 0

    nc = bacc.Bacc("TRN2", target_bir_lowering=False, debug=False)

    # ---- DRAM tensors -------------------------------------------------
    idx_d = nc.dram_tensor("idx", [P, ntile], I32, kind="ExternalInput")
    emb_d = nc.dram_tensor("emb", [V, D], F32, kind="ExternalInput")
    w_d = {}
    for dname in ("f", "b"):
        for c, (k0, ks) in enumerate(DCH):
            w_d[dname, c] = nc.dram_tensor(f"w{dname}{c}", [ks, 384], BF16,
                                           kind="ExternalInput")
    uf_d = nc.dram_tensor("uf", [H, 384], BF16, kind="ExternalInput")
    ub_d = nc.dram_tensor("ub", [H, 384], BF16, kind="ExternalInput")
    brf_d = nc.dram_tensor("brf", [H, 1], F32, kind="ExternalInput")
    brb_d = nc.dram_tensor("brb", [H, 1], F32, kind="ExternalInput")
    h0_d = nc.dram_tensor("h0", [H, 2 * Bc], F32, kind="ExternalInput")
    h0b_d = nc.dram_tensor("h0b16", [H, 2 * Bc], BF16, kind="ExternalInput")

    out_d = nc.dram_tensor("out", [Bc, T, 2 * H], F32, kind="ExternalOutput")
    hf_d = nc.dram_tensor("hf", [H, Bc], F32, kind="ExternalOutput")
    hb_d = nc.dram_tensor("hb", [H, Bc], F32, kind="ExternalOutput")

    with tile.TileContext(nc) as tc:
        from contextlib import ExitStack
        with ExitStack() as ctx:
            cst = ctx.enter_context(tc.tile_pool(name="cst", bufs=1))
            xtp = ctx.enter_context(tc.tile_pool(name="xtp", bufs=max(1, ntile // max(1, RW // P))))
            oap = ctx.enter_context(tc.tile_pool(name="oap", bufs=1))
            xrw = ctx.enter_context(tc.tile_pool(name="xrw", bufs=3))
            pwp = ctx.enter_context(tc.tile_pool(name="pwp", bufs=3))
            chp = ctx.enter_context(tc.tile_pool(name="chp", bufs=2, space="PSUM"))
            chx = ctx.enter_context(tc.tile_pool(name="chx", bufs=1, space="PSUM"))
            scp = ctx.enter_context(tc.tile_pool(name="scp", bufs=1, space="PSUM"))
            tpp = ctx.enter_context(tc.tile_pool(name="tpp", bufs=2, space="PSUM"))

            # Bacc's compile passes split multi-sem waits into
            # EventSemaphore instructions (HW allows 1 wait per inst).
            def mm(out, lhsT, rhs, **kw):
                return nc.tensor.matmul(out, lhsT=lhsT, rhs=rhs, **kw)

            def tr(out, in_, identity):
                return nc.tensor.transpose(out, in_, identity)

            # ---- constants / weights into SBUF ------------------------
            ident = cst.tile([P, P], F32, tag="ident")
            make_identity(nc, ident[:])

            idx_sb = cst.tile([P, ntile], I32, tag="idx")
            nc.sync.dma_start(idx_sb[:], idx_d[:])

            w_sb = {}
            for dname in ("f", "b"):
                for c, (k0, ks) in enumerate(DCH):
                    t = cst.tile([ks, 384], BF16, tag=f"w{dname}{c}")
                    nc.sync.dma_start(t[:], w_d[dname, c][:])
                    w_sb[dname, c] = t
            uf = cst.tile([H, 384], BF16, tag="uf")
            nc.sync.dma_start(uf[:], uf_d[:])
            ub = cst.tile([H, 384], BF16, tag="ub")
            nc.sync.dma_start(ub[:], ub_d[:])
            brf = cst.tile([H, 1], F32, tag="brf")
            nc.sync.dma_start(brf[:], brf_d[:])
            brb = cst.tile([H, 1], F32, tag="brb")
            nc.sync.dma_start(brb[:], brb_d[:])
            h0sb = cst.tile([H, 2 * Bc], F32, tag="h0")
            nc.sync.dma_start(h0sb[:], h0_d[:])
            h0bf = cst.tile([H, 2 * Bc], BF16, tag="h0b16")
            nc.sync.dma_start(h0bf[:], h0b_d[:])
            hbfp = ctx.enter_context(tc.tile_pool(name="hbfp", bufs=2))

            # HAM warmup: ~5us of dense matmuls so the PE clock-gate
            # opens (K=8/8); recurrence-phase gaps are ~1us < the ~3.4us
            # MID window, so it stays warm afterwards.
            wup = chp.tile([P, 4 * RW], F32, tag="ch")
            for i in range(12):
                mm(wup[:, 0:384], lhsT=uf[:, 0:128], rhs=uf[:],
                   start=True, stop=True)

            # big persistent buffers
            out_all = oap.tile([P, 2 * NT], F32, tag="out_all")
            oa = out_all[:]
            oa_p = oa.ap[0]  # partition dim [stride, 128]

            def hstate3(s):
                """Combined state [128, 2, 16]: dir-f cols b*T+s, dir-b cols
                NT + b*T + (T-1-s).  Affine per fixed s."""
                if s < 0:
                    return h0sb[:].rearrange("p (d b) -> p d b", d=2)
                off_f = s
                off_b = NT + (T - 1 - s)
                return AP(oa.tensor, oa.offset + off_f,
                          [oa_p, [off_b - off_f, 2], [T, Bc]])

            def hstate_dir(s, d):
                if s < 0:
                    return h0sb[:, d * Bc:(d + 1) * Bc]
                off = s if d == 0 else NT + (T - 1 - s)
                return AP(oa.tensor, oa.offset + off, [oa_p, [T, Bc]])

            # ---- gather + transpose x into feature-major tiles ---------
            # x row n = t*Bc + b ; 256-col tile j covers n in [256j, 256j+256)
            ntile2 = ntile // XW
            xt_of = {0: {}, 1: {}, 2: {}}
            order = []
            lo, hi = 0, ntile2 - 1
            while lo <= hi:
                order.append(lo)
                if hi != lo:
                    order.append(hi)
                lo += 1
                hi -= 1

            for j in order:
                xtile = {}
                for c, (k0, ks) in enumerate(DCH):
                    xtile[c] = xtp.tile([ks, XW * P], BF16, tag=f"xt{c}", name=f"xt{c}_{j}")
                    xt_of[c][j] = xtile[c]
                for half in range(XW):
                    k = j * XW + half
                    xr = xrw.tile([P, 304], F32, tag="xr")
                    nc.vector.memset(xr[:, 300:301], 1.0)
                    nc.gpsimd.indirect_dma_start(
                        out=xr[:, 0:D], out_offset=None, in_=emb_d[:],
                        in_offset=IndirectOffsetOnAxis(ap=idx_sb[:, k:k + 1], axis=0))
                    for c, (k0, ks) in enumerate(DCH):
                        tp = tpp.tile([P, P], F32, tag="tp")
                        tr(tp[0:ks, 0:P], xr[:, k0:k0 + ks], ident[:])
                        nc.scalar.copy(xtile[c][:, half * P:(half + 1) * P],
                                       tp[0:ks, 0:P])

            # ---- psum chunk fill --------------------------------------
            # zr tile regions (RW=256 cols each): 0: z_f  1: z_b  2: r_f  3: r_b
            # xh tile regions:                    0: xh_f 1: xh_b
            # PSUM has_written semantics: a start=True matmul clears the
            # accumulate-bits of its ENTIRE bank, so exactly one start=True
            # per bank per chunk generation (the first matmul touching it);
            # everything else start=False (unwritten elements get
            # overwritten, written ones accumulate — this is what lets the
            # per-step recurrent matmuls accumulate later).
            def fill_chunk(kc):
                pz = chp.tile([P, 4 * RW], F32, tag="ch")
                pz4 = pz[:].rearrange("p (r q) -> p r q", r=4)
                px = chx.tile([P, 2 * RW], F32, tag="cx")
                px2 = px[:].rearrange("p (r q) -> p r q", r=2)
                for di, (dname, ktile) in enumerate((("f", kc),
                                                     ("b", ntile2 - 1 - kc))):
                    for g in range(3):
                        for c, (k0, ks) in enumerate(DCH):
                            if g == 2:
                                dst = px2[:, di, :]
                                first = (di == 0) and c == 0
                            else:
                                dst = pz4[:, 2 * g + di, :]
                                first = (di == 0) and c == 0
                            mm(dst,
                               lhsT=w_sb[dname, c][:, g * 128:(g + 1) * 128],
                               rhs=xt_of[c][ktile][:],
                               start=first, stop=(c == 2),
                               skip_group_check=True)
                return pz, px

            # ---- recurrence -------------------------------------------
            pz4 = None
            px2 = None
            for s in range(T):
                kc, dt = divmod(s, TC)
                dtb = TC - 1 - dt
                if dt == 0:
                    pz, px = fill_chunk(kc)
                    pz4 = pz[:].rearrange("p (r q) -> p r q", r=4)
                    px2 = px[:].rearrange("p (r q) -> p r q", r=2)

                hp = h0bf[:] if s == 0 else h_bf[:]
                hp_f = hp[:, 0:Bc]
                hp_b = hp[:, Bc:2 * Bc]
                sc = scp.tile([P, 2 * Bc], F32, tag="sc")

                mm(pz4[:, 0, dt * Bc:(dt + 1) * Bc], lhsT=uf[:, 0:128],
                   rhs=hp_f, start=False, stop=True, skip_group_check=True)
                mm(pz4[:, 2, dt * Bc:(dt + 1) * Bc], lhsT=uf[:, 128:256],
                   rhs=hp_f, start=False, stop=True, skip_group_check=True)
                mm(sc[:, 0:Bc], lhsT=uf[:, 256:384],
                   rhs=hp_f, start=True, stop=True)
                mm(pz4[:, 1, dtb * Bc:(dtb + 1) * Bc], lhsT=ub[:, 0:128],
                   rhs=hp_b, start=False, stop=True, skip_group_check=True)
                mm(pz4[:, 3, dtb * Bc:(dtb + 1) * Bc], lhsT=ub[:, 128:256],
                   rhs=hp_b, start=False, stop=True, skip_group_check=True)
                mm(sc[:, Bc:2 * Bc], lhsT=ub[:, 256:384],
                   rhs=hp_b, start=True, stop=True)

                # zr layout: [z_f | r_f | z_b | r_b] (16 each)
                zr = pwp.tile([P, 4 * Bc], F32, tag="zr")
                zr4 = zr[:].rearrange("p (r q) -> p r q", r=4)
                nc.scalar.activation(
                    zr4[:, 0:2, :],
                    pz[:].rearrange("p (a r q) -> p a r q", a=2, r=2)[:, :, 0,
                                                                     dt * Bc:(dt + 1) * Bc],
                    AF.Sigmoid)
                nc.scalar.activation(
                    zr4[:, 2:4, :],
                    pz[:].rearrange("p (a r q) -> p a r q", a=2, r=2)[:, :, 1,
                                                                     dtb * Bc:(dtb + 1) * Bc],
                    AF.Sigmoid)

                tt = pwp.tile([P, 2 * Bc], F32, tag="tt")
                nc.vector.scalar_tensor_tensor(tt[:, 0:Bc], in0=sc[:, 0:Bc],
                                               scalar=brf[:, 0:1], in1=zr[:, Bc:2 * Bc],
                                               op0=ALU.add, op1=ALU.mult)
                nc.vector.scalar_tensor_tensor(tt[:, Bc:2 * Bc], in0=sc[:, Bc:2 * Bc],
                                               scalar=brb[:, 0:1], in1=zr[:, 3 * Bc:4 * Bc],
                                               op0=ALU.add, op1=ALU.mult)

                u = pwp.tile([P, 2 * Bc], F32, tag="u")
                nc.vector.tensor_tensor(u[:, 0:Bc], tt[:, 0:Bc],
                                        px2[:, 0, dt * Bc:(dt + 1) * Bc], op=ALU.add)
                nc.vector.tensor_tensor(u[:, Bc:2 * Bc], tt[:, Bc:2 * Bc],
                                        px2[:, 1, dtb * Bc:(dtb + 1) * Bc], op=ALU.add)

                hh = pwp.tile([P, 2 * Bc], F32, tag="hh")
                nc.scalar.activation(hh[:], u[:], AF.Tanh)

                # h' = z*h + (1-z)*hh ; w=1-z and zh=z*h run in the tanh
                # shadow so only v and h' sit on the serial chain.
                z3 = zr[:].rearrange("p (d r q) -> p d r q", d=2, r=2)[:, :, 0, :]
                w = pwp.tile([P, 2 * Bc], F32, tag="w")
                w2 = w[:].rearrange("p (d b) -> p d b", d=2)
                nc.vector.tensor_scalar(w2, z3, -1.0, 1.0, ALU.mult, ALU.add)
                zh = pwp.tile([P, 2 * Bc], F32, tag="zh")
                zh2 = zh[:].rearrange("p (d b) -> p d b", d=2)
                nc.vector.tensor_tensor(zh2, z3, hstate3(s - 1), op=ALU.mult)

                v = pwp.tile([P, 2 * Bc], F32, tag="v")
                nc.vector.tensor_tensor(v[:], w[:], hh[:], op=ALU.mult)
                h_bf = hbfp.tile([P, 2 * Bc], BF16, tag="hbf")
                nc.vector.tensor_tensor(h_bf[:], v[:], zh[:], op=ALU.add)
                nc.vector.tensor_tensor(hstate3(s), v[:].rearrange("p (d b) -> p d b", d=2),
                                        zh2, op=ALU.add)

                # ---- output flush -------------------------------------
                if (s + 1) % FB == 0:
                    j = s // FB
                    jb = T // FB - 1 - j
                    for b in range(Bc):
                        tp = tpp.tile([P, P], F32, tag="tp")
                        tr(tp[0:FB, 0:P],
                           oa[:, b * T + j * FB: b * T + (j + 1) * FB], ident[:])
                        ob = pwp.tile([P, P], F32, tag="ob")
                        if b % 2 == 0:
                            nc.scalar.copy(ob[0:FB, 0:P], tp[0:FB, 0:P])
                        else:
                            nc.vector.tensor_copy(ob[0:FB, 0:P], tp[0:FB, 0:P])
                        nc.sync.dma_start(out_d[b, j * FB:(j + 1) * FB, 0:H],
                                          ob[0:FB, 0:P])
                        tp2 = tpp.tile([P, P], F32, tag="tp")
                        tr(tp2[0:FB, 0:P],
                           oa[:, NT + b * T + jb * FB: NT + b * T + (jb + 1) * FB],
                           ident[:])
                        ob2 = pwp.tile([P, P], F32, tag="ob")
                        if b % 2 == 1:
                            nc.scalar.copy(ob2[0:FB, 0:P], tp2[0:FB, 0:P])
                        else:
                            nc.vector.tensor_copy(ob2[0:FB, 0:P], tp2[0:FB, 0:P])
                        nc.sync.dma_start(out_d[b, jb * FB:(jb + 1) * FB, H:2 * H],
                                          ob2[0:FB, 0:P])

            # ---- final states -----------------------------------------
            stg_f = pwp.tile([P, Bc], F32, tag="stg")
            nc.vector.tensor_copy(stg_f[:], hstate_dir(T - 1, 0))
            nc.sync.dma_start(hf_d[:], stg_f[:])
            stg_b = pwp.tile([P, Bc], F32, tag="stg")
            nc.vector.tensor_copy(stg_b[:], hstate_dir(T - 1, 1))
            nc.sync.dma_start(hb_d[:], stg_b[:])

    nc.finalize()
    return nc


@functools.lru_cache(maxsize=2)
def _cached_program(T, Bc):
    return build_program(T, Bc)


def _host_inputs(encoder_input, state_fwd, state_back, emb, W_f, U_f, b_f,
                 W_b, U_b, b_b, T, Bc, n_cores):
    """Build per-core in_maps (plain numpy, layout prep only)."""
    ntile = T * Bc // P
    emb = np.ascontiguousarray(emb, dtype=np.float32)

    def w_aug(W, b2):
        bias = b2[0] + np.concatenate([b2[1, :256], np.zeros(128, np.float32)])
        return np.concatenate([W, bias[None, :].astype(np.float32)], axis=0)

    wf_a = w_aug(W_f, b_f)   # [301, 384]
    wb_a = w_aug(W_b, b_b)
    import ml_dtypes
    bf16 = ml_dtypes.bfloat16
    shared = {
        "emb": emb,
        "wf0": np.ascontiguousarray(wf_a[0:128]).astype(bf16),
        "wf1": np.ascontiguousarray(wf_a[128:256]).astype(bf16),
        "wf2": np.ascontiguousarray(wf_a[256:301]).astype(bf16),
        "wb0": np.ascontiguousarray(wb_a[0:128]).astype(bf16),
        "wb1": np.ascontiguousarray(wb_a[128:256]).astype(bf16),
        "wb2": np.ascontiguousarray(wb_a[256:301]).astype(bf16),
        "uf": np.ascontiguousarray(U_f, dtype=np.float32).astype(bf16),
        "ub": np.ascontiguousarray(U_b, dtype=np.float32).astype(bf16),
        "brf": np.ascontiguousarray(b_f[1, 256:384].reshape(H, 1)),
        "brb": np.ascontiguousarray(b_b[1, 256:384].reshape(H, 1)),
    }
    in_maps = []
    for c in range(n_cores):
        sl = slice(c * Bc, (c + 1) * Bc)
        enc = encoder_input[sl]                      # [Bc, T]
        idx_flat = np.ascontiguousarray(enc.T).reshape(-1)   # n = t*Bc + b
        idx_sb = np.ascontiguousarray(idx_flat.reshape(ntile, P).T,
                                      dtype=np.int32)        # [128, ntile]
        h0 = np.concatenate([state_fwd[sl].T, state_back[sl].T],
                            axis=1).astype(np.float32)       # [128, 2*Bc]
        in_maps.append(dict(shared, idx=idx_sb, h0=h0, h0b16=h0.astype(bf16)))
    return in_maps


def run_sharded(encoder_input, state_fwd, state_back, emb, W_f, U_f, b_f,
                W_b, U_b, b_b, T=None, Bc=None, n_cores=None, trace=False):
    B = encoder_input.shape[0]
    T = T or encoder_input.shape[1]
    n_cores = n_cores or N_CORES
    Bc = Bc or B // n_cores
    nc = _cached_program(T, Bc)
    in_maps = _host_inputs(encoder_input, state_fwd, state_back, emb,
                           W_f, U_f, b_f, W_b, U_b, b_b, T, Bc, n_cores)
    res = run_bass_kernel_spmd(nc, in_maps, core_ids=list(range(n_cores)),
                               trace=trace)
    outs = res.results
    enc_out = np.concatenate([o["out"] for o in outs], axis=0)
    h_f = np.concatenate([o["hf"].T for o in outs], axis=0)
    h_b = np.concatenate([o["hb"].T for o in outs], axis=0)
    return (enc_out, h_f, h_b), res


def kernel(encoder_input, state_fwd, state_back, emb, W_f, U_f, b_f,
           W_b, U_b, b_b):
    (enc_out, h_f, h_b), _ = run_sharded(
        np.asarray(encoder_input), np.asarray(state_fwd, dtype=np.float32),
        np.asarray(state_back, dtype=np.float32), np.asarray(emb, dtype=np.float32),
        np.asarray(W_f, dtype=np.float32), np.asarray(U_f, dtype=np.float32),
        np.asarray(b_f, dtype=np.float32), np.asarray(W_b, dtype=np.float32),
        np.asarray(U_b, dtype=np.float32), np.asarray(b_b, dtype=np.float32))
    return enc_out, h_f, h_b


# revision 13
# speedup vs baseline: 2.3136x; 1.1328x over previous
"""Bidirectional GRU encoder (Keras reset_after=True) on 8 Trainium2 NeuronCores.

Problem (hardcoded): B=128, T=512, V=32000, D=300, H=128, fp32.
  x = emb[encoder_input]                       # [B,T,300] gather
  out_f, h_f = GRU_fwd(x);  out_b, h_b = GRU_bwd(x reversed)
  return concat([out_f, out_b], -1), h_f, h_b

Sharding: data-parallel over batch, 16 rows per core; both directions run on
every core.  Inside a core everything is feature-major [H=128 partitions,
batch] so the recurrent state feeds the gate matmuls with no transposes.

Device-side plan per core (one Tile program, fully unrolled over T):
  - embedding rows are gathered 128 at a time with indirect DMA into
    [128, 304] row tiles, PE-transposed into x^T tiles [d_chunk, 128]
    (d chunks 128/128/45; the 45th row of chunk 2 is a constant 1 used to
    fold all additive biases into the input-projection matmul),
  - input projections x@W+b for both directions are computed chunk-by-chunk
    (8 timesteps at a time) straight into PSUM; the per-step recurrent
    matmuls U.T @ h accumulate into the same PSUM regions (z, r gates),
  - per step: sigmoid (z,r) -> (rec_h + b_rh) * r -> + x_h -> tanh ->
    h' = hh + z*(h - hh), written directly into the big SBUF output buffer
    which doubles as the recurrent state,
  - every 128 steps finished output columns are PE-transposed and DMA'd to
    DRAM as [b, t, h].
"""

import os
import sys
import functools

import numpy as np

for _p in ("/opt/trn_rl_repo",):
    if _p not in sys.path and os.path.isdir(_p):
        sys.path.insert(0, _p)

import concourse.bass as bass
import concourse.mybir as mybir
import concourse.tile as tile
from concourse import bacc
from concourse.bass import AP, IndirectOffsetOnAxis
from concourse.bass_utils import run_bass_kernel_spmd
from concourse.masks import make_identity

F32 = mybir.dt.float32
BF16 = mybir.dt.bfloat16
I32 = mybir.dt.int32
AF = mybir.ActivationFunctionType
ALU = mybir.AluOpType

# Problem constants
B_FULL, T_FULL, V, D, H = 128, 512, 32000, 300, 128
N_CORES = 8
P = 128  # partitions


def build_program(T: int, Bc: int):
    """Build the single-core Bass/Tile program (SPMD across cores)."""
    TC = min(16, T)         # timesteps per psum chunk
    NT = T * Bc             # total (t, b) positions per direction
    ntile = NT // P         # 128-row tiles of gathered x
    RW = Bc * TC            # region width in psum chunk (256)
    XW = RW // P            # 128-col x tiles per chunk (2)
    nch = T // TC           # number of chunks
    FB = min(128, T)        # output flush block (timesteps)
    DCH = [(0, 128), (128, 128), (256, 45)]  # D-chunks incl. bias row
    assert T % TC == 0 and T % FB == 0 and NT % P == 0 and ntile % XW == 0

    nc = bacc.Bacc("TRN2", target_bir_lowering=False, debug=False)

    # ---- DRAM tensors -------------------------------------------------
    idx_d = nc.dram_tensor("idx", [P, ntile], I32, kind="ExternalInput")
    emb_d = nc.dram_tensor("emb", [V, D], F32, kind="ExternalInput")
    w_d = {}
    for dname in ("f", "b"):
        for c, (k0, ks) in enumerate(DCH):
            w_d[dname, c] = nc.dram_tensor(f"w{dname}{c}", [ks, 384], BF16,
                                           kind="ExternalInput")
    uf_d = nc.dram_tensor("uf", [H, 384], BF16, kind="ExternalInput")
    ub_d = nc.dram_tensor("ub", [H, 384], BF16, kind="ExternalInput")
    brfr_d = nc.dram_tensor("brfr", [1, H], BF16, kind="ExternalInput")
    brbr_d = nc.dram_tensor("brbr", [1, H], BF16, kind="ExternalInput")
    h0_d = nc.dram_tensor("h0", [H, 2 * Bc], F32, kind="ExternalInput")
    h0b_d = nc.dram_tensor("h0b16", [H, 2 * Bc], BF16, kind="ExternalInput")

    out_d = nc.dram_tensor("out", [Bc, T, 2 * H], F32, kind="ExternalOutput")
    hf_d = nc.dram_tensor("hf", [H, Bc], F32, kind="ExternalOutput")
    hb_d = nc.dram_tensor("hb", [H, Bc], F32, kind="ExternalOutput")

    with tile.TileContext(nc) as tc:
        from contextlib import ExitStack
        with ExitStack() as ctx:
            cst = ctx.enter_context(tc.tile_pool(name="cst", bufs=1))
            xtp = ctx.enter_context(tc.tile_pool(name="xtp", bufs=max(1, ntile // max(1, RW // P))))
            oap = ctx.enter_context(tc.tile_pool(name="oap", bufs=1))
            xrw = ctx.enter_context(tc.tile_pool(name="xrw", bufs=3))
            pwp = ctx.enter_context(tc.tile_pool(name="pwp", bufs=3))
            chp = ctx.enter_context(tc.tile_pool(name="chp", bufs=2, space="PSUM"))
            chx = ctx.enter_context(tc.tile_pool(name="chx", bufs=1, space="PSUM"))
            scp = ctx.enter_context(tc.tile_pool(name="scp", bufs=1, space="PSUM"))
            tpp = ctx.enter_context(tc.tile_pool(name="tpp", bufs=2, space="PSUM"))

            # Bacc's compile passes split multi-sem waits into
            # EventSemaphore instructions (HW allows 1 wait per inst).
            def mm(out, lhsT, rhs, **kw):
                return nc.tensor.matmul(out, lhsT=lhsT, rhs=rhs, **kw)

            def tr(out, in_, identity):
                return nc.tensor.transpose(out, in_, identity)

            # ---- constants / weights into SBUF ------------------------
            ident = cst.tile([P, P], F32, tag="ident")
            make_identity(nc, ident[:])

            idx_sb = cst.tile([P, ntile], I32, tag="idx")
            nc.sync.dma_start(idx_sb[:], idx_d[:])

            w_sb = {}
            for dname in ("f", "b"):
                for c, (k0, ks) in enumerate(DCH):
                    t = cst.tile([ks, 384], BF16, tag=f"w{dname}{c}")
                    nc.sync.dma_start(t[:], w_d[dname, c][:])
                    w_sb[dname, c] = t
            uf = cst.tile([H, 384], BF16, tag="uf")
            nc.sync.dma_start(uf[:], uf_d[:])
            ub = cst.tile([H, 384], BF16, tag="ub")
            nc.sync.dma_start(ub[:], ub_d[:])
            brfr = cst.tile([1, H], BF16, tag="brfr")
            nc.sync.dma_start(brfr[:], brfr_d[:])
            brbr = cst.tile([1, H], BF16, tag="brbr")
            nc.sync.dma_start(brbr[:], brbr_d[:])
            ones_row = cst.tile([1, RW], BF16, tag="ones_row")
            nc.vector.memset(ones_row[:], 1.0)
            h0sb = cst.tile([H, 2 * Bc], F32, tag="h0")
            nc.sync.dma_start(h0sb[:], h0_d[:])
            h0bf = cst.tile([H, 2 * Bc], BF16, tag="h0b16")
            nc.sync.dma_start(h0bf[:], h0b_d[:])
            hbfp = ctx.enter_context(tc.tile_pool(name="hbfp", bufs=2))

            # HAM warmup: ~5us of dense matmuls so the PE clock-gate
            # opens (K=8/8); recurrence-phase gaps are ~1us < the ~3.4us
            # MID window, so it stays warm afterwards.
            wup = chp.tile([P, 4 * RW], F32, tag="ch")
            for i in range(12):
                mm(wup[:, 0:384], lhsT=uf[:, 0:128], rhs=uf[:],
                   start=True, stop=True)

            # big persistent buffers
            out_all = oap.tile([P, 2 * NT], F32, tag="out_all")
            oa = out_all[:]
            oa_p = oa.ap[0]  # partition dim [stride, 128]

            def hstate3(s):
                """Combined state [128, 2, 16]: dir-f cols b*T+s, dir-b cols
                NT + b*T + (T-1-s).  Affine per fixed s."""
                if s < 0:
                    return h0sb[:].rearrange("p (d b) -> p d b", d=2)
                off_f = s
                off_b = NT + (T - 1 - s)
                return AP(oa.tensor, oa.offset + off_f,
                          [oa_p, [off_b - off_f, 2], [T, Bc]])

            def hstate_dir(s, d):
                if s < 0:
                    return h0sb[:, d * Bc:(d + 1) * Bc]
                off = s if d == 0 else NT + (T - 1 - s)
                return AP(oa.tensor, oa.offset + off, [oa_p, [T, Bc]])

            # ---- gather + transpose x into feature-major tiles ---------
            # x row n = t*Bc + b ; 256-col tile j covers n in [256j, 256j+256)
            ntile2 = ntile // XW
            xt_of = {0: {}, 1: {}, 2: {}}
            order = []
            lo, hi = 0, ntile2 - 1
            while lo <= hi:
                order.append(lo)
                if hi != lo:
                    order.append(hi)
                lo += 1
                hi -= 1

            for j in order:
                xtile = {}
                for c, (k0, ks) in enumerate(DCH):
                    xtile[c] = xtp.tile([ks, XW * P], BF16, tag=f"xt{c}", name=f"xt{c}_{j}")
                    xt_of[c][j] = xtile[c]
                for half in range(XW):
                    k = j * XW + half
                    xr = xrw.tile([P, 304], F32, tag="xr")
                    nc.vector.memset(xr[:, 300:301], 1.0)
                    nc.gpsimd.indirect_dma_start(
                        out=xr[:, 0:D], out_offset=None, in_=emb_d[:],
                        in_offset=IndirectOffsetOnAxis(ap=idx_sb[:, k:k + 1], axis=0))
                    for c, (k0, ks) in enumerate(DCH):
                        tp = tpp.tile([P, P], F32, tag="tp")
                        tr(tp[0:ks, 0:P], xr[:, k0:k0 + ks], ident[:])
                        nc.scalar.copy(xtile[c][:, half * P:(half + 1) * P],
                                       tp[0:ks, 0:P])

            # ---- psum chunk fill --------------------------------------
            # zr tile regions (RW=256 cols each): 0: z_f  1: z_b  2: r_f  3: r_b
            # xh tile regions:                    0: xh_f 1: xh_b
            # PSUM has_written semantics: a start=True matmul clears the
            # accumulate-bits of its ENTIRE bank, so exactly one start=True
            # per bank per chunk generation (the first matmul touching it);
            # everything else start=False (unwritten elements get
            # overwritten, written ones accumulate — this is what lets the
            # per-step recurrent matmuls accumulate later).
            def fill_chunk(kc):
                pz = chp.tile([P, 4 * RW], F32, tag="ch")
                pz4 = pz[:].rearrange("p (r q) -> p r q", r=4)
                px = chx.tile([P, 2 * RW], F32, tag="cx")
                px2 = px[:].rearrange("p (r q) -> p r q", r=2)
                sc = scp.tile([P, 2 * RW], F32, tag="sc")
                mm(sc[:, 0:RW], lhsT=brfr[:], rhs=ones_row[:],
                   start=True, stop=True, skip_group_check=True)
                mm(sc[:, RW:2 * RW], lhsT=brbr[:], rhs=ones_row[:],
                   start=False, stop=True, skip_group_check=True)
                for di, (dname, ktile) in enumerate((("f", kc),
                                                     ("b", ntile2 - 1 - kc))):
                    for g in range(3):
                        for c, (k0, ks) in enumerate(DCH):
                            if g == 2:
                                dst = px2[:, di, :]
                                first = (di == 0) and c == 0
                            else:
                                dst = pz4[:, 2 * g + di, :]
                                first = (di == 0) and c == 0
                            mm(dst,
                               lhsT=w_sb[dname, c][:, g * 128:(g + 1) * 128],
                               rhs=xt_of[c][ktile][:],
                               start=first, stop=(c == 2),
                               skip_group_check=True)
                return pz, px, sc

            # ---- recurrence -------------------------------------------
            pz = px = sc = None
            for s in range(T):
                kc, dt = divmod(s, TC)
                dtb = TC - 1 - dt
                if dt == 0:
                    pz, px, sc = fill_chunk(kc)
                dstride = RW + (dtb - dt) * Bc   # f-part -> b-part col stride
                pz_a = pz[:]
                px_a = px[:]
                sc_a = sc[:]

                hp = h0bf[:] if s == 0 else h_bf[:]
                hp_f = hp[:, 0:Bc]
                hp_b = hp[:, Bc:2 * Bc]

                mm(AP(pz_a.tensor, pz_a.offset + dt * Bc, [pz_a.ap[0], [1, Bc]]),
                   lhsT=uf[:, 0:128],
                   rhs=hp_f, start=False, stop=True, skip_group_check=True)
                mm(AP(pz_a.tensor, pz_a.offset + 2 * RW + dt * Bc, [pz_a.ap[0], [1, Bc]]),
                   lhsT=uf[:, 128:256],
                   rhs=hp_f, start=False, stop=True, skip_group_check=True)
                mm(AP(sc_a.tensor, sc_a.offset + dt * Bc, [sc_a.ap[0], [1, Bc]]),
                   lhsT=uf[:, 256:384],
                   rhs=hp_f, start=False, stop=True, skip_group_check=True)
                mm(AP(pz_a.tensor, pz_a.offset + RW + dtb * Bc, [pz_a.ap[0], [1, Bc]]),
                   lhsT=ub[:, 0:128],
                   rhs=hp_b, start=False, stop=True, skip_group_check=True)
                mm(AP(pz_a.tensor, pz_a.offset + 3 * RW + dtb * Bc, [pz_a.ap[0], [1, Bc]]),
                   lhsT=ub[:, 128:256],
                   rhs=hp_b, start=False, stop=True, skip_group_check=True)
                mm(AP(sc_a.tensor, sc_a.offset + RW + dtb * Bc, [sc_a.ap[0], [1, Bc]]),
                   lhsT=ub[:, 256:384],
                   rhs=hp_b, start=False, stop=True, skip_group_check=True)

                # zr layout: [z_f | z_b | r_f | r_b] (16 each); one sigmoid
                # covers all four via a 4D AP (the f->b stride folds both the
                # region offset and the dt->dtb shift).
                zr = pwp.tile([P, 4 * Bc], F32, tag="zr")
                sig_in = AP(pz_a.tensor, pz_a.offset + dt * Bc,
                            [pz_a.ap[0], [dstride, 2], [2 * RW, 2], [1, Bc]])
                sig_out = AP(zr[:].tensor, zr[:].offset,
                             [zr[:].ap[0], [Bc, 2], [2 * Bc, 2], [1, Bc]])
                nc.scalar.activation(sig_out, sig_in, AF.Sigmoid)

                tt = pwp.tile([P, 2 * Bc], F32, tag="tt")
                t_in0 = AP(sc_a.tensor, sc_a.offset + dt * Bc,
                           [sc_a.ap[0], [dstride, 2], [1, Bc]])
                nc.vector.tensor_tensor(
                    tt[:].rearrange("p (d b) -> p d b", d=2), t_in0,
                    zr[:, 2 * Bc:4 * Bc].rearrange("p (d b) -> p d b", d=2),
                    op=ALU.mult)

                u = pwp.tile([P, 2 * Bc], F32, tag="u")
                u_in1 = AP(px_a.tensor, px_a.offset + dt * Bc,
                           [px_a.ap[0], [dstride, 2], [1, Bc]])
                nc.vector.tensor_tensor(
                    u[:].rearrange("p (d b) -> p d b", d=2),
                    tt[:].rearrange("p (d b) -> p d b", d=2), u_in1, op=ALU.add)

                hh = pwp.tile([P, 2 * Bc], F32, tag="hh")
                nc.scalar.activation(hh[:], u[:], AF.Tanh)

                # h' = z*h + (1-z)*hh ; w=1-z and zh=z*h run in the tanh
                # shadow so only v and h' sit on the serial chain.
                z3 = zr[:, 0:2 * Bc].rearrange("p (d b) -> p d b", d=2)
                w = pwp.tile([P, 2 * Bc], F32, tag="w")
                w2 = w[:].rearrange("p (d b) -> p d b", d=2)
                nc.vector.tensor_scalar(w2, z3, -1.0, 1.0, ALU.mult, ALU.add)
                zh = pwp.tile([P, 2 * Bc], F32, tag="zh")
                zh2 = zh[:].rearrange("p (d b) -> p d b", d=2)
                nc.vector.tensor_tensor(zh2, z3, hstate3(s - 1), op=ALU.mult)

                v = pwp.tile([P, 2 * Bc], F32, tag="v")
                nc.vector.tensor_tensor(v[:], w[:], hh[:], op=ALU.mult)
                h_bf = hbfp.tile([P, 2 * Bc], BF16, tag="hbf")
                nc.vector.tensor_tensor(h_bf[:], v[:], zh[:], op=ALU.add)
                nc.vector.tensor_tensor(hstate3(s), v[:].rearrange("p (d b) -> p d b", d=2),
                                        zh2, op=ALU.add)

                # ---- output flush -------------------------------------
                if (s + 1) % FB == 0:
                    j = s // FB
                    jb = T // FB - 1 - j
                    for b in range(Bc):
                        tp = tpp.tile([P, P], F32, tag="tp")
                        tr(tp[0:FB, 0:P],
                           oa[:, b * T + j * FB: b * T + (j + 1) * FB], ident[:])
                        ob = pwp.tile([P, P], F32, tag="ob")
                        if b % 2 == 0:
                            nc.scalar.copy(ob[0:FB, 0:P], tp[0:FB, 0:P])
                        else:
                            nc.vector.tensor_copy(ob[0:FB, 0:P], tp[0:FB, 0:P])
                        nc.sync.dma_start(out_d[b, j * FB:(j + 1) * FB, 0:H],
                                          ob[0:FB, 0:P])
                        tp2 = tpp.tile([P, P], F32, tag="tp")
                        tr(tp2[0:FB, 0:P],
                           oa[:, NT + b * T + jb * FB: NT + b * T + (jb + 1) * FB],
                           ident[:])
                        ob2 = pwp.tile([P, P], F32, tag="ob")
                        if b % 2 == 1:
                            nc.scalar.copy(ob2[0:FB, 0:P], tp2[0:FB, 0:P])
                        else:
                            nc.vector.tensor_copy(ob2[0:FB, 0:P], tp2[0:FB, 0:P])
                        nc.sync.dma_start(out_d[b, jb * FB:(jb + 1) * FB, H:2 * H],
                                          ob2[0:FB, 0:P])

            # ---- final states -----------------------------------------
            stg_f = pwp.tile([P, Bc], F32, tag="stg")
            nc.vector.tensor_copy(stg_f[:], hstate_dir(T - 1, 0))
            nc.sync.dma_start(hf_d[:], stg_f[:])
            stg_b = pwp.tile([P, Bc], F32, tag="stg")
            nc.vector.tensor_copy(stg_b[:], hstate_dir(T - 1, 1))
            nc.sync.dma_start(hb_d[:], stg_b[:])

    nc.finalize()
    return nc


@functools.lru_cache(maxsize=2)
def _cached_program(T, Bc):
    return build_program(T, Bc)


def _host_inputs(encoder_input, state_fwd, state_back, emb, W_f, U_f, b_f,
                 W_b, U_b, b_b, T, Bc, n_cores):
    """Build per-core in_maps (plain numpy, layout prep only)."""
    ntile = T * Bc // P
    emb = np.ascontiguousarray(emb, dtype=np.float32)

    def w_aug(W, b2):
        bias = b2[0] + np.concatenate([b2[1, :256], np.zeros(128, np.float32)])
        return np.concatenate([W, bias[None, :].astype(np.float32)], axis=0)

    wf_a = w_aug(W_f, b_f)   # [301, 384]
    wb_a = w_aug(W_b, b_b)
    import ml_dtypes
    bf16 = ml_dtypes.bfloat16
    shared = {
        "emb": emb,
        "wf0": np.ascontiguousarray(wf_a[0:128]).astype(bf16),
        "wf1": np.ascontiguousarray(wf_a[128:256]).astype(bf16),
        "wf2": np.ascontiguousarray(wf_a[256:301]).astype(bf16),
        "wb0": np.ascontiguousarray(wb_a[0:128]).astype(bf16),
        "wb1": np.ascontiguousarray(wb_a[128:256]).astype(bf16),
        "wb2": np.ascontiguousarray(wb_a[256:301]).astype(bf16),
        "uf": np.ascontiguousarray(U_f, dtype=np.float32).astype(bf16),
        "ub": np.ascontiguousarray(U_b, dtype=np.float32).astype(bf16),
        "brfr": np.ascontiguousarray(b_f[1, 256:384].reshape(1, H)).astype(bf16),
        "brbr": np.ascontiguousarray(b_b[1, 256:384].reshape(1, H)).astype(bf16),
    }
    in_maps = []
    for c in range(n_cores):
        sl = slice(c * Bc, (c + 1) * Bc)
        enc = encoder_input[sl]                      # [Bc, T]
        idx_flat = np.ascontiguousarray(enc.T).reshape(-1)   # n = t*Bc + b
        idx_sb = np.ascontiguousarray(idx_flat.reshape(ntile, P).T,
                                      dtype=np.int32)        # [128, ntile]
        h0 = np.concatenate([state_fwd[sl].T, state_back[sl].T],
                            axis=1).astype(np.float32)       # [128, 2*Bc]
        in_maps.append(dict(shared, idx=idx_sb, h0=h0, h0b16=h0.astype(bf16)))
    return in_maps


def run_sharded(encoder_input, state_fwd, state_back, emb, W_f, U_f, b_f,
                W_b, U_b, b_b, T=None, Bc=None, n_cores=None, trace=False):
    B = encoder_input.shape[0]
    T = T or encoder_input.shape[1]
    n_cores = n_cores or N_CORES
    Bc = Bc or B // n_cores
    nc = _cached_program(T, Bc)
    in_maps = _host_inputs(encoder_input, state_fwd, state_back, emb,
                           W_f, U_f, b_f, W_b, U_b, b_b, T, Bc, n_cores)
    res = run_bass_kernel_spmd(nc, in_maps, core_ids=list(range(n_cores)),
                               trace=trace)
    outs = res.results
    enc_out = np.concatenate([o["out"] for o in outs], axis=0)
    h_f = np.concatenate([o["hf"].T for o in outs], axis=0)
    h_b = np.concatenate([o["hb"].T for o in outs], axis=0)
    return (enc_out, h_f, h_b), res


def kernel(encoder_input, state_fwd, state_back, emb, W_f, U_f, b_f,
           W_b, U_b, b_b):
    (enc_out, h_f, h_b), _ = run_sharded(
        np.asarray(encoder_input), np.asarray(state_fwd, dtype=np.float32),
        np.asarray(state_back, dtype=np.float32), np.asarray(emb, dtype=np.float32),
        np.asarray(W_f, dtype=np.float32), np.asarray(U_f, dtype=np.float32),
        np.asarray(b_f, dtype=np.float32), np.asarray(W_b, dtype=np.float32),
        np.asarray(U_b, dtype=np.float32), np.asarray(b_b, dtype=np.float32))
    return enc_out, h_f, h_b
